# revision 1
# baseline (speedup 1.0000x reference)
"""Multi-head attention (B=2, T=4096, C=768, H=12, Dk=64) on 8 trn2 NeuronCores.

Sharding: core c -> batch b = c//4, head-group g = c%4 (3 heads each).
Megatron-style: each core computes qkv projection for its 3 heads, full
attention for those heads, and a row-parallel partial of the output
projection. Host sums the 4 partials per batch (+ bias, folded into the
g==0 core's partial on device).

Device algorithm (per core), everything fp32:
  - qkT[c, t] feature-major via matmul(lhsT=Wqk_cols, rhs=xT) with
    column packing [q0 q1 | k0 k1 | q2 k2] so head0 lives on SBUF
    partitions 0-63 and head1 on 64-127 (natural PE row-tiling pairs),
    plus a swapped duplicate slot o3 = [k2 | q2] so head2 pairs across
    alternating tk-blocks.
  - V token-major [t, 64] per head with an appended ones column ->
    attention matmul also accumulates the softmax denominator.
  - attention in S^T layout: ST[tk,tq-blk] = KT^T@QT, exp on ACT engine
    (scale=1/8 fused into the activation), OT^T[dv,tq] += V_aug^T@expST.
  - normalize with DVE reciprocal + gpsimd partition broadcast.
  - out projection from OT^T (feature-major) with Wout rows.
"""

import os
import sys
from contextlib import ExitStack

import numpy as np

for _p in ("/opt/trn_rl_repo", "/root/.axon_site/_ro/trn_rl_repo"):
    if os.path.isdir(_p) and _p not in sys.path:
        sys.path.append(_p)

import concourse.bass as bass
import concourse.mybir as mybir
import concourse.tile as tile
from concourse import bacc
from concourse.bass import ts
from concourse.bass_utils import run_bass_kernel_spmd

F32 = mybir.dt.float32
F32R = mybir.dt.float32r
F16 = mybir.dt.float16

B, T, C = 2, 4096, 768
H, DK = 12, 64
N_CORES = 8
HPC = 3  # heads per core
GQ = 512  # q-block (matmul free dim)
NTQ = T // GQ  # 8 q-blocks
NTK = T // 128  # 32 tk-blocks


def _build_program(debug_taps=False):
    nc = bacc.Bacc("TRN2", target_bir_lowering=False, debug=False)

    xT = nc.dram_tensor("xT", [C, T], F32R, kind="ExternalInput").ap()
    wqk = nc.dram_tensor("wqk", [C, 384], F32R, kind="ExternalInput").ap()
    bqk = nc.dram_tensor("bqk", [384], F32, kind="ExternalInput").ap()
    wv = nc.dram_tensor("wv", [C, 192], F32R, kind="ExternalInput").ap()
    bv = nc.dram_tensor("bv", [192], F32, kind="ExternalInput").ap()
    wout = nc.dram_tensor("wout", [192, C], F16, kind="ExternalInput").ap()
    bout = nc.dram_tensor("bout", [C], F32, kind="ExternalInput").ap()
    ones = nc.dram_tensor("ones", [NTK * HPC], F16, kind="ExternalInput").ap()
    y = nc.dram_tensor("y", [T, C], F32, kind="ExternalOutput").ap()

    xT3 = xT.rearrange("(o p) t -> p o t", p=128)  # [128, 6, 4096]
    y3 = y.rearrange("(n p) e -> p n e", p=128)  # [128, 32, 768]

    dbg = {}
    if debug_taps:
        for name, shape in [
            ("dbg_qkT", [128, 4, 512]),
            ("dbg_vaug", [128, 3, 65]),
            ("dbg_est", [128, 2, 512]),
            ("dbg_ot", [65, 512]),
            ("dbg_rc", [1, 512]),
            ("dbg_rb", [64, 512]),
            ("dbg_otn", [64, 512]),
        ]:
            dbg[name] = nc.dram_tensor(name, shape, F32, kind="ExternalOutput").ap()

    with tile.TileContext(nc) as tc, ExitStack() as ctx:
        sb = ctx.enter_context(tc.tile_pool(name="persist", bufs=1))

        # --- weights / biases ---
        wqk_sb = sb.tile([128, 6, 384], F32R)
        nc.sync.dma_start(wqk_sb[:], wqk.rearrange("(o p) c -> p o c", p=128))
        wv_sb = sb.tile([128, 6, 192], F32R)
        nc.sync.dma_start(wv_sb[:], wv.rearrange("(o p) c -> p o c", p=128))
        wout_sb = sb.tile([64, 3, C], F16)
        nc.sync.dma_start(wout_sb[:], wout.rearrange("(h p) e -> p h e", p=64))
        bqk_sb = sb.tile([128, 3], F32)
        nc.sync.dma_start(bqk_sb[:], bqk.rearrange("(o p) -> p o", p=128))
        bv_bc = sb.tile([128, 192], F32)
        nc.sync.dma_start(bv_bc[:], bv[None, :].to_broadcast((128, 192)))
        bout_bc = sb.tile([128, C], F32)
        nc.sync.dma_start(bout_bc[:], bout[None, :].to_broadcast((128, C)))

        # --- persistent activations ---
        # qkT slots: o0=[q0|q1] o1=[k0|k1] o2=[q2|k2] o3=[k2|q2]
        qkT = sb.tile([128, 4, T], F16)
        # V per tk-block per head, token-major, with ones col at [.., 64]
        vaug = sb.tile([128, NTK, HPC, 65], F16)
        nc.sync.dma_start(
            vaug[:, :, :, 64:65], ones[None, :].to_broadcast((128, NTK * HPC))
        )

        # --- prologue: qkv projections ---
        with (
            tc.tile_pool(name="psA", bufs=2, space="PSUM") as psA,
            tc.tile_pool(name="xin", bufs=2) as xin,
        ):
            for tb in range(NTQ):  # qk proj over 512-col blocks of t
                xt = xin.tile([128, 6, GQ], F32R, tag="xqk")
                nc.sync.dma_start(xt[:], xT3[:, :, ts(tb, GQ)])
                for cb in range(3):
                    ps = psA.tile([128, GQ], F32, tag="qk")
                    for d in range(6):
                        nc.tensor.matmul(
                            ps[:],
                            (wqk_sb[:, d, ts(cb, 128)]),
                            (xt[:, d, :]),
                            start=(d == 0),
                            stop=(d == 5),
                        )
                    nc.vector.tensor_scalar_add(
                        qkT[:, cb, ts(tb, GQ)], ps[:], bqk_sb[:, cb : cb + 1]
                    )
                # o3 = swap halves of o2 (k2|q2)
                nc.sync.dma_start(qkT[0:64, 3, ts(tb, GQ)], qkT[64:128, 2, ts(tb, GQ)])
                nc.sync.dma_start(qkT[64:128, 3, ts(tb, GQ)], qkT[0:64, 2, ts(tb, GQ)])

            for tb in range(NTK):  # v proj over 128-row blocks of t
                xv = xin.tile([128, 6, 128], F32R, tag="xv")
                nc.sync.dma_start(xv[:], xT3[:, :, ts(tb, 128)])
                psv = psA.tile([128, 192], F32, tag="v")
                for d in range(6):
                    nc.tensor.matmul(
                        psv[:],
                        (xv[:, d, :]),
                        (wv_sb[:, d, :]),
                        start=(d == 0),
                        stop=(d == 5),
                    )
                nc.vector.tensor_tensor(
                    vaug[:, tb, :, 0:64],
                    psv[:].rearrange("p (h d) -> p h d", d=64),
                    bv_bc[:].rearrange("p (h d) -> p h d", d=64),
                    mybir.AluOpType.add,
                )

        # --- attention + output projection ---
        with (
            tc.tile_pool(name="psST", bufs=2, space="PSUM") as psST,
            tc.tile_pool(name="psOT", bufs=3, space="PSUM") as psOT,
            tc.tile_pool(name="psY", bufs=1, space="PSUM") as psY,
            tc.tile_pool(name="estp", bufs=3) as estp,
            tc.tile_pool(name="otp", bufs=2) as otp,
            tc.tile_pool(name="smallp", bufs=3) as smallp,
            tc.tile_pool(name="yp", bufs=2) as yp,
            tc.tile_pool(name="dramp", bufs=3, space="DRAM") as dramp,
        ):

            def normalize(ps_ot, ot_dst, tap=False):
                rc = smallp.tile([1, GQ], F32, tag="rc")
                nc.vector.reciprocal(rc[:], ps_ot[64:65, :])
                dn = dramp.tile([GQ], F32, tag="dn")
                nc.sync.dma_start(dn[:], rc[:])
                rb = smallp.tile([64, GQ], F32, tag="rb")
                nc.sync.dma_start(rb[:], dn[None, :].to_broadcast((64, GQ)))
                nc.vector.tensor_tensor(
                    ot_dst, ps_ot[0:64, :], rb[:], mybir.AluOpType.mult
                )
                if tap:
                    ots = smallp.tile([65, GQ], F32, tag="dbg_ots")
                    nc.vector.tensor_copy(ots[:], ps_ot[:])
                    nc.sync.dma_start(dbg["dbg_ot"], ots[:])
                    nc.sync.dma_start(dbg["dbg_rc"], rc[:])
                    nc.sync.dma_start(dbg["dbg_rb"], rb[:])
                    nc.sync.dma_start(dbg["dbg_otn"], ot_dst)

            for tq in range(NTQ):
                ot_tile = otp.tile([64, HPC, GQ], F16, tag="ot_sb")

                # -- heads 0,1 (paired on PE rows lo/hi) --
                ps_ot0 = psOT.tile([65, GQ], F32, tag="ot")
                ps_ot1 = psOT.tile([65, GQ], F32, tag="ot")
                for g in range(NTK):
                    st = psST.tile([128, 2, GQ], F32, tag="st")
                    nc.tensor.matmul(
                        st[:, 0, :],
                        (qkT[0:64, 1, ts(g, 128)]),
                        (qkT[0:64, 0, ts(tq, GQ)]),
                        start=True,
                        stop=True,
                    )
                    nc.tensor.matmul(
                        st[:, 1, :],
                        (qkT[64:128, 1, ts(g, 128)]),
                        (qkT[64:128, 0, ts(tq, GQ)]),
                        start=True,
                        stop=True,
                    )
                    est = estp.tile([128, 2, GQ], F16, tag="est")
                    nc.scalar.activation(
                        est[:], st[:], mybir.ActivationFunctionType.Exp, scale=0.125
                    )
                    if debug_taps and tq == 0 and g == 0:
                        nc.sync.dma_start(dbg["dbg_est"], est[:])
                        nc.sync.dma_start(dbg["dbg_qkT"], qkT[:, :, 0:512])
                        nc.sync.dma_start(dbg["dbg_vaug"], vaug[:, 0, :, :])
                    nc.tensor.matmul(
                        ps_ot0[:],
                        (vaug[:, g, 0, :]),
                        (est[:, 0, :]),
                        start=(g == 0),
                        stop=(g == NTK - 1),
                    )
                    nc.tensor.matmul(
                        ps_ot1[:],
                        (vaug[:, g, 1, :]),
                        (est[:, 1, :]),
                        start=(g == 0),
                        stop=(g == NTK - 1),
                    )
                normalize(ps_ot0, ot_tile[:, 0, :], tap=(debug_taps and tq == 0))
                normalize(ps_ot1, ot_tile[:, 1, :])

                # -- head 2 (paired across even/odd tk-blocks) --
                ps_ot2 = psOT.tile([65, GQ], F32, tag="ot")
                for g2 in range(NTK // 2):
                    ge, go = 2 * g2, 2 * g2 + 1
                    st = psST.tile([128, 2, GQ], F32, tag="st")
                    nc.tensor.matmul(
                        st[:, 0, :],
                        (qkT[0:64, 3, ts(ge, 128)]),
                        (qkT[0:64, 2, ts(tq, GQ)]),
                        start=True,
                        stop=True,
                    )
                    nc.tensor.matmul(
                        st[:, 1, :],
                        (qkT[64:128, 2, ts(go, 128)]),
                        (qkT[64:128, 3, ts(tq, GQ)]),
                        start=True,
                        stop=True,
                    )
                    est = estp.tile([128, 2, GQ], F16, tag="est")
                    nc.scalar.activation(
                        est[:], st[:], mybir.ActivationFunctionType.Exp, scale=0.125
                    )
                    nc.tensor.matmul(
                        ps_ot2[:],
                        (vaug[:, ge, 2, :]),
                        (est[:, 0, :]),
                        start=(g2 == 0),
                        stop=False,
                    )
                    nc.tensor.matmul(
                        ps_ot2[:],
                        (vaug[:, go, 2, :]),
                        (est[:, 1, :]),
                        start=False,
                        stop=(g2 == NTK // 2 - 1),
                    )
                normalize(ps_ot2, ot_tile[:, 2, :])

                # -- output projection for this q-block --
                for tsub in range(GQ // 128):
                    y_sb = yp.tile([128, C], F32, tag="y_sb")
                    for nh in range(2):
                        py = psY.tile([128, 384], F32, tag="y")
                        for h in range(HPC):
                            nc.tensor.matmul(
                                py[:],
                                (ot_tile[:, h, ts(tsub, 128)]),
                                (wout_sb[:, h, ts(nh, 384)]),
                                start=(h == 0),
                                stop=(h == HPC - 1),
                            )
                        nc.vector.tensor_tensor(
                            y_sb[:, ts(nh, 384)],
                            py[:],
                            bout_bc[:, ts(nh, 384)],
                            mybir.AluOpType.add,
                        )
                    nc.sync.dma_start(y3[:, tq * (GQ // 128) + tsub, :], y_sb[:])

    nc.compile()
    return nc


_PROGRAM = None


def _get_program():
    global _PROGRAM
    if _PROGRAM is None:
        _PROGRAM = _build_program()
    return _PROGRAM


def _make_in_maps(x, W_qkv, b_qkv, W_out, b_out):
    x = np.asarray(x, dtype=np.float32)
    W_qkv = np.asarray(W_qkv, dtype=np.float32)
    b_qkv = np.asarray(b_qkv, dtype=np.float32)
    W_out = np.asarray(W_out, dtype=np.float32)
    b_out = np.asarray(b_out, dtype=np.float32)

    xT_b = [np.ascontiguousarray(x[b].T) for b in range(B)]
    in_maps = []
    for c in range(N_CORES):
        b, g = divmod(c, 4)
        h0 = HPC * g

        def qcol(h):
            return slice(h * DK, (h + 1) * DK)

        def kcol(h):
            return slice(C + h * DK, C + (h + 1) * DK)

        wqk_c = np.concatenate(
            [
                W_qkv[:, qcol(h0)],
                W_qkv[:, qcol(h0 + 1)],
                W_qkv[:, kcol(h0)],
                W_qkv[:, kcol(h0 + 1)],
                W_qkv[:, qcol(h0 + 2)],
                W_qkv[:, kcol(h0 + 2)],
            ],
            axis=1,
        )
        bqk_c = np.concatenate(
            [
                b_qkv[qcol(h0)],
                b_qkv[qcol(h0 + 1)],
                b_qkv[kcol(h0)],
                b_qkv[kcol(h0 + 1)],
                b_qkv[qcol(h0 + 2)],
                b_qkv[kcol(h0 + 2)],
            ]
        )
        vs = slice(2 * C + h0 * DK, 2 * C + (h0 + HPC) * DK)
        in_maps.append(
            {
                "xT": np.ascontiguousarray(xT_b[b]),
                "wqk": np.ascontiguousarray(wqk_c),
                "bqk": np.ascontiguousarray(bqk_c),
                "wv": np.ascontiguousarray(W_qkv[:, vs]),
                "bv": np.ascontiguousarray(b_qkv[vs]),
                "wout": np.ascontiguousarray(W_out[h0 * DK : (h0 + HPC) * DK, :]).astype(np.float16),
                "bout": (b_out if g == 0 else np.zeros_like(b_out)).copy(),
                "ones": np.ones(NTK * HPC, dtype=np.float16),
            }
        )
    return in_maps


def _assemble(results):
    out = np.zeros((B, T, C), dtype=np.float32)
    for c in range(N_CORES):
        out[c // 4] += results[c]["y"]
    return out


def kernel_run(inputs, trace=False):
    """Returns (full_output [B,T,C] fp32, exec_time_ns or None)."""
    nc = _get_program()
    in_maps = _make_in_maps(**inputs)
    res = run_bass_kernel_spmd(
        nc, in_maps, core_ids=list(range(N_CORES)), trace=trace
    )
    return _assemble(res.results), res.exec_time_ns


def kernel(**inputs):
    out, _ = kernel_run(inputs)
    return out



# revision 2
# speedup vs baseline: 1.2361x; 1.2361x over previous
"""Multi-head attention (B=2, T=4096, C=768, H=12, Dk=64) on 8 trn2 NeuronCores.

Sharding: core c -> batch b = c//4, head-group g = c%4 (3 heads each).
Megatron-style: each core computes qkv projection for its 3 heads, full
attention for those heads, and a row-parallel partial of the output
projection. Host sums the 4 partials per batch (+ bias, folded into the
g==0 core's partial on device).

v2 changes vs baseline (865us):
  - exp split across ACT (exact, scale=0.125 fused) and DVE (Schraudolph:
    est_bits_i16 = round(S*K + B), bit-viewed as f16 -> ~3% max rel err on
    the approx half; validated end-to-end ~2e-3) - removes the ACT
    serialization that dominated the baseline.
  - x loaded once (v-proj runs off the same 512-wide x tiles as qk-proj,
    via an ACT-side f16 cast), halving x HBM traffic.
  - v-proj in f16 (1 cyc/row vs fp32r's 4 at free-dim 192).
  - qk bias copy moved to ACT (idle in prologue).
  - software-pipelined attention loop (ST issued 2 iterations ahead of AV)
    so the PE never waits on exp latency.
  - out-projection pipelined one tq-block behind attention so the
    normalize DMA roundtrip is off the critical path.
"""

import os
import sys
from contextlib import ExitStack

import numpy as np

for _p in ("/opt/trn_rl_repo", "/root/.axon_site/_ro/trn_rl_repo"):
    if os.path.isdir(_p) and _p not in sys.path:
        sys.path.append(_p)

import concourse.bass as bass
import concourse.mybir as mybir
import concourse.tile as tile
from concourse import bacc
from concourse.bass import ts
from concourse.bass_utils import run_bass_kernel_spmd

F32 = mybir.dt.float32
F32R = mybir.dt.float32r
F16 = mybir.dt.float16
I16 = mybir.dt.int16

B, T, C = 2, 4096, 768
H, DK = 12, 64
N_CORES = 8
HPC = 3  # heads per core
GQ = 512  # q-block (matmul free dim)
NTQ = T // GQ  # 8 q-blocks
NTK = T // 128  # 32 tk-blocks

# Schraudolph f16-bit exp: exp(s/8) ~ bits_f16(round(s*K + B)), mean-zero
# offset so the approx blocks carry no systematic bias vs the exact blocks
# sharing the same softmax denominator.
K_SCH = 1024.0 * float(np.log2(np.e)) / 8.0
B_SCH = 15308.5


def _build_program():
    nc = bacc.Bacc("TRN2", target_bir_lowering=False, debug=False)

    xT = nc.dram_tensor("xT", [C, T], F32R, kind="ExternalInput").ap()
    wqk = nc.dram_tensor("wqk", [C, 384], F32R, kind="ExternalInput").ap()
    bqk = nc.dram_tensor("bqk", [384], F32, kind="ExternalInput").ap()
    wv = nc.dram_tensor("wv", [C, 192], F16, kind="ExternalInput").ap()
    bv = nc.dram_tensor("bv", [192], F32, kind="ExternalInput").ap()
    wout = nc.dram_tensor("wout", [192, C], F16, kind="ExternalInput").ap()
    bout = nc.dram_tensor("bout", [C], F32, kind="ExternalInput").ap()
    ones = nc.dram_tensor("ones", [NTK * HPC], F16, kind="ExternalInput").ap()
    y = nc.dram_tensor("y", [T, C], F32, kind="ExternalOutput").ap()

    xT3 = xT.rearrange("(o p) t -> p o t", p=128)  # [128, 6, 4096]
    y3 = y.rearrange("(n p) e -> p n e", p=128)  # [128, 32, 768]

    with tile.TileContext(nc) as tc, ExitStack() as ctx:
        sb = ctx.enter_context(tc.tile_pool(name="persist", bufs=1))

        # --- weights / biases ---
        wqk_sb = sb.tile([128, 6, 384], F32R)
        nc.sync.dma_start(wqk_sb[:], wqk.rearrange("(o p) c -> p o c", p=128))
        wv_sb = sb.tile([128, 6, 192], F16)
        nc.sync.dma_start(wv_sb[:], wv.rearrange("(o p) c -> p o c", p=128))
        wout_sb = sb.tile([64, 3, C], F16)
        nc.sync.dma_start(wout_sb[:], wout.rearrange("(h p) e -> p h e", p=64))
        bqk_sb = sb.tile([128, 3], F32)
        nc.sync.dma_start(bqk_sb[:], bqk.rearrange("(o p) -> p o", p=128))
        bv_bc = sb.tile([128, 192], F32)
        nc.sync.dma_start(bv_bc[:], bv[None, :].to_broadcast((128, 192)))
        bout_bc = sb.tile([128, C], F32)
        nc.sync.dma_start(bout_bc[:], bout[None, :].to_broadcast((128, C)))

        # --- persistent activations ---
        # qkT slots: o0=[q0|q1] o1=[k0|k1] o2=[q2|k2] o3=[k2|q2]
        qkT = sb.tile([128, 4, T], F16)
        # V per tk-block per head, token-major, with ones col at [.., 64]
        vaug = sb.tile([128, NTK, HPC, 65], F16)
        nc.sync.dma_start(
            vaug[:, :, :, 64:65], ones[None, :].to_broadcast((128, NTK * HPC))
        )

        # --- prologue: qkv projections, x loaded once per 512-block ---
        with (
            tc.tile_pool(name="psQK", bufs=2, space="PSUM") as psQK,
            tc.tile_pool(name="psV", bufs=2, space="PSUM") as psV,
            tc.tile_pool(name="xin", bufs=2) as xin,
            tc.tile_pool(name="x16p", bufs=2) as x16p,
        ):
            for tb in range(NTQ):
                xt = xin.tile([128, 6, GQ], F32R, tag="xqk")
                nc.sync.dma_start(xt[:], xT3[:, :, ts(tb, GQ)])
                # f16 copy for the v-projection (ACT is idle in prologue)
                xt16 = x16p.tile([128, 6, GQ], F16, tag="x16")
                nc.scalar.activation(
                    xt16[:], xt[:], mybir.ActivationFunctionType.Copy
                )
                for cb in range(3):
                    ps = psQK.tile([128, GQ], F32, tag="qk")
                    for d in range(6):
                        nc.tensor.matmul(
                            ps[:],
                            (wqk_sb[:, d, ts(cb, 128)]),
                            (xt[:, d, :]),
                            start=(d == 0),
                            stop=(d == 5),
                        )
                    # bias-add + f16 copy on ACT (Identity, per-partition bias)
                    nc.scalar.activation(
                        qkT[:, cb, ts(tb, GQ)],
                        ps[:],
                        mybir.ActivationFunctionType.Identity,
                        bias=bqk_sb[:, cb : cb + 1],
                    )
                # o3 = swap halves of o2 (k2|q2)
                nc.sync.dma_start(qkT[0:64, 3, ts(tb, GQ)], qkT[64:128, 2, ts(tb, GQ)])
                nc.sync.dma_start(qkT[64:128, 3, ts(tb, GQ)], qkT[0:64, 2, ts(tb, GQ)])

                # v-projection from the same x tile (f16), 128-token sub-blocks
                for sub in range(GQ // 128):
                    g = tb * (GQ // 128) + sub
                    psv = psV.tile([128, 192], F32, tag="v")
                    for d in range(6):
                        nc.tensor.matmul(
                            psv[:],
                            (xt16[:, d, ts(sub, 128)]),
                            (wv_sb[:, d, :]),
                            start=(d == 0),
                            stop=(d == 5),
                        )
                    nc.vector.tensor_tensor(
                        vaug[:, g, :, 0:64],
                        psv[:].rearrange("p (h d) -> p h d", d=64),
                        bv_bc[:].rearrange("p (h d) -> p h d", d=64),
                        mybir.AluOpType.add,
                    )

        # --- attention + output projection ---
        with (
            tc.tile_pool(name="psST", bufs=2, space="PSUM") as psST,
            tc.tile_pool(name="psOT", bufs=3, space="PSUM") as psOT,
            tc.tile_pool(name="psY", bufs=1, space="PSUM") as psY,
            tc.tile_pool(name="estA", bufs=3) as estA,
            tc.tile_pool(name="estD", bufs=3) as estD,
            tc.tile_pool(name="otp", bufs=2) as otp,
            tc.tile_pool(name="smallp", bufs=3) as smallp,
            tc.tile_pool(name="yp", bufs=2) as yp,
            tc.tile_pool(name="dramp", bufs=3, space="DRAM") as dramp,
        ):

            def expst(st, idx):
                """exp(st*0.125): 3/5 exact-ACT, 2/5 Schraudolph-DVE."""
                if idx % 5 not in (1, 3):
                    est = estA.tile([128, 2, GQ], F16, tag="estA")
                    nc.scalar.activation(
                        est[:], st[:], mybir.ActivationFunctionType.Exp, scale=0.125
                    )
                    return est
                esti = estD.tile([128, 2, GQ], I16, tag="estD")
                nc.vector.tensor_scalar(
                    esti[:],
                    st[:],
                    K_SCH,
                    B_SCH,
                    mybir.AluOpType.mult,
                    mybir.AluOpType.add,
                )
                return esti.bitcast(F16)

            def normalize(ps_ot, ot_dst):
                rc = smallp.tile([1, GQ], F32, tag="rc")
                nc.vector.reciprocal(rc[:], ps_ot[64:65, :])
                dn = dramp.tile([GQ], F32, tag="dn")
                nc.sync.dma_start(dn[:], rc[:])
                rb = smallp.tile([64, GQ], F32, tag="rb")
                nc.sync.dma_start(rb[:], dn[None, :].to_broadcast((64, GQ)))
                nc.vector.tensor_tensor(
                    ot_dst, ps_ot[0:64, :], rb[:], mybir.AluOpType.mult
                )

            def out_proj(tq, ot_tile):
                for tsub in range(GQ // 128):
                    y_sb = yp.tile([128, C], F32, tag="y_sb")
                    for nh in range(2):
                        py = psY.tile([128, 384], F32, tag="y")
                        for h in range(HPC):
                            nc.tensor.matmul(
                                py[:],
                                (ot_tile[:, h, ts(tsub, 128)]),
                                (wout_sb[:, h, ts(nh, 384)]),
                                start=(h == 0),
                                stop=(h == HPC - 1),
                            )
                        nc.vector.tensor_tensor(
                            y_sb[:, ts(nh, 384)],
                            py[:],
                            bout_bc[:, ts(nh, 384)],
                            mybir.AluOpType.add,
                        )
                    nc.sync.dma_start(y3[:, tq * (GQ // 128) + tsub, :], y_sb[:])

            prev = None  # (tq, ot_tile) pending out-projection
            for tq in range(NTQ):
                ot_tile = otp.tile([64, HPC, GQ], F16, tag="ot_sb")

                # -- phase A: heads 0,1 (paired on PE rows lo/hi) --
                # software-pipelined: ST/exp issued 2 iterations ahead of AV
                ps_ot0 = psOT.tile([65, GQ], F32, tag="ot")
                ps_ot1 = psOT.tile([65, GQ], F32, tag="ot")
                ests = {}
                for i in range(NTK + 2):
                    if i < NTK:
                        st = psST.tile([128, 2, GQ], F32, tag="st")
                        nc.tensor.matmul(
                            st[:, 0, :],
                            (qkT[0:64, 1, ts(i, 128)]),
                            (qkT[0:64, 0, ts(tq, GQ)]),
                            start=True,
                            stop=True,
                        )
                        nc.tensor.matmul(
                            st[:, 1, :],
                            (qkT[64:128, 1, ts(i, 128)]),
                            (qkT[64:128, 0, ts(tq, GQ)]),
                            start=True,
                            stop=True,
                        )
                        ests[i] = expst(st, i)
                    if i >= 2:
                        g = i - 2
                        est = ests.pop(g)
                        nc.tensor.matmul(
                            ps_ot0[:],
                            (vaug[:, g, 0, :]),
                            (est[:, 0, :]),
                            start=(g == 0),
                            stop=(g == NTK - 1),
                        )
                        nc.tensor.matmul(
                            ps_ot1[:],
                            (vaug[:, g, 1, :]),
                            (est[:, 1, :]),
                            start=(g == 0),
                            stop=(g == NTK - 1),
                        )
                normalize(ps_ot0, ot_tile[:, 0, :])
                normalize(ps_ot1, ot_tile[:, 1, :])

                # -- phase B: head 2 (paired across even/odd tk-blocks) --
                ps_ot2 = psOT.tile([65, GQ], F32, tag="ot")
                for i in range(NTK // 2 + 2):
                    if i < NTK // 2:
                        ge, go = 2 * i, 2 * i + 1
                        st = psST.tile([128, 2, GQ], F32, tag="st")
                        nc.tensor.matmul(
                            st[:, 0, :],
                            (qkT[0:64, 3, ts(ge, 128)]),
                            (qkT[0:64, 2, ts(tq, GQ)]),
                            start=True,
                            stop=True,
                        )
                        nc.tensor.matmul(
                            st[:, 1, :],
                            (qkT[64:128, 2, ts(go, 128)]),
                            (qkT[64:128, 3, ts(tq, GQ)]),
                            start=True,
                            stop=True,
                        )
                        ests[i] = expst(st, i)
                    if i >= 2:
                        g2 = i - 2
                        ge, go = 2 * g2, 2 * g2 + 1
                        est = ests.pop(g2)
                        nc.tensor.matmul(
                            ps_ot2[:],
                            (vaug[:, ge, 2, :]),
                            (est[:, 0, :]),
                            start=(g2 == 0),
                            stop=False,
                        )
                        nc.tensor.matmul(
                            ps_ot2[:],
                            (vaug[:, go, 2, :]),
                            (est[:, 1, :]),
                            start=False,
                            stop=(g2 == NTK // 2 - 1),
                        )
                # out-projection pipelined one tq behind (its ot is long ready)
                if prev is not None:
                    out_proj(*prev)
                normalize(ps_ot2, ot_tile[:, 2, :])
                prev = (tq, ot_tile)
            out_proj(*prev)

    nc.compile()
    return nc


_PROGRAM = None


def _get_program():
    global _PROGRAM
    if _PROGRAM is None:
        _PROGRAM = _build_program()
    return _PROGRAM


def _make_in_maps(x, W_qkv, b_qkv, W_out, b_out):
    x = np.asarray(x, dtype=np.float32)
    W_qkv = np.asarray(W_qkv, dtype=np.float32)
    b_qkv = np.asarray(b_qkv, dtype=np.float32)
    W_out = np.asarray(W_out, dtype=np.float32)
    b_out = np.asarray(b_out, dtype=np.float32)

    xT_b = [np.ascontiguousarray(x[b].T) for b in range(B)]
    in_maps = []
    for c in range(N_CORES):
        b, g = divmod(c, 4)
        h0 = HPC * g

        def qcol(h):
            return slice(h * DK, (h + 1) * DK)

        def kcol(h):
            return slice(C + h * DK, C + (h + 1) * DK)

        wqk_c = np.concatenate(
            [
                W_qkv[:, qcol(h0)],
                W_qkv[:, qcol(h0 + 1)],
                W_qkv[:, kcol(h0)],
                W_qkv[:, kcol(h0 + 1)],
                W_qkv[:, qcol(h0 + 2)],
                W_qkv[:, kcol(h0 + 2)],
            ],
            axis=1,
        )
        bqk_c = np.concatenate(
            [
                b_qkv[qcol(h0)],
                b_qkv[qcol(h0 + 1)],
                b_qkv[kcol(h0)],
                b_qkv[kcol(h0 + 1)],
                b_qkv[qcol(h0 + 2)],
                b_qkv[kcol(h0 + 2)],
            ]
        )
        vs = slice(2 * C + h0 * DK, 2 * C + (h0 + HPC) * DK)
        in_maps.append(
            {
                "xT": np.ascontiguousarray(xT_b[b]),
                "wqk": np.ascontiguousarray(wqk_c),
                "bqk": np.ascontiguousarray(bqk_c),
                "wv": np.ascontiguousarray(W_qkv[:, vs]).astype(np.float16),
                "bv": np.ascontiguousarray(b_qkv[vs]),
                "wout": np.ascontiguousarray(
                    W_out[h0 * DK : (h0 + HPC) * DK, :]
                ).astype(np.float16),
                "bout": (b_out if g == 0 else np.zeros_like(b_out)).copy(),
                "ones": np.ones(NTK * HPC, dtype=np.float16),
            }
        )
    return in_maps


def _assemble(results):
    out = np.zeros((B, T, C), dtype=np.float32)
    for c in range(N_CORES):
        out[c // 4] += results[c]["y"]
    return out


def kernel_run(inputs, trace=False):
    """Returns (full_output [B,T,C] fp32, exec_time_ns or None)."""
    nc = _get_program()
    in_maps = _make_in_maps(**inputs)
    res = run_bass_kernel_spmd(
        nc, in_maps, core_ids=list(range(N_CORES)), trace=trace
    )
    return _assemble(res.results), res.exec_time_ns


def kernel(**inputs):
    out, _ = kernel_run(inputs)
    return out


# revision 4
# speedup vs baseline: 1.6815x; 1.3603x over previous
"""Multi-head attention (B=2, T=4096, C=768, H=12, Dk=64) on 8 trn2 NeuronCores.

Sharding: core c -> batch b = c//4, head-group g = c%4 (3 heads each).
Megatron-style: each core computes qkv projection for its 3 heads, full
attention for those heads, and a row-parallel partial of the output
projection. Host sums the 4 partials per batch (+ bias, folded into the
g==0 core's partial on device).

v3:
  - exp split 3/5 ACT (exact) / 2/5 DVE (Schraudolph f16-bit, mean-zero
    offset; end-to-end rel err ~7e-3 vs the 2e-2 gate).
  - x & W_qk in f16, x loaded once in 2 big DMAs (sync + gpsimd queues).
  - ones column written by DVE memset (was a 104us descriptor-storm DMA).
  - reciprocal_approx_fast for softmax denominators (5x cheaper, 51 ULP).
  - normalize DMA roundtrips & qkT o3-swaps on the gpsimd DMA queue.
  - software-pipelined attention (ST 2 ahead of AV) + out-projection
    pipelined one tq behind so normalize latency is off the critical path.
"""

import os
import sys
from contextlib import ExitStack

import numpy as np

for _p in ("/opt/trn_rl_repo", "/root/.axon_site/_ro/trn_rl_repo"):
    if os.path.isdir(_p) and _p not in sys.path:
        sys.path.append(_p)

import concourse.bass as bass
import concourse.mybir as mybir
import concourse.tile as tile
from concourse import bacc
from concourse.bass import ts
from concourse.bass_utils import run_bass_kernel_spmd

F32 = mybir.dt.float32
F16 = mybir.dt.float16
I16 = mybir.dt.int16

B, T, C = 2, 4096, 768
H, DK = 12, 64
N_CORES = 8
HPC = 3  # heads per core
GQ = 512  # q-block (matmul free dim)
NTQ = T // GQ  # 8 q-blocks
NTK = T // 128  # 32 tk-blocks

# Schraudolph f16-bit exp: exp(s/8) ~ bits_f16(round(s*K + B)), mean-zero
# offset so the approx blocks carry no systematic bias vs the exact blocks
# sharing the same softmax denominator.
K_SCH = 1024.0 * float(np.log2(np.e)) / 8.0
B_SCH = 15308.5


def _build_program():
    nc = bacc.Bacc("TRN2", target_bir_lowering=False, debug=False)

    xT = nc.dram_tensor("xT", [C, T], F16, kind="ExternalInput").ap()
    wqk = nc.dram_tensor("wqk", [C, 384], F16, kind="ExternalInput").ap()
    bqk = nc.dram_tensor("bqk", [384], F32, kind="ExternalInput").ap()
    wv = nc.dram_tensor("wv", [C, 192], F16, kind="ExternalInput").ap()
    bv = nc.dram_tensor("bv", [192], F32, kind="ExternalInput").ap()
    wout = nc.dram_tensor("wout", [192, C], F16, kind="ExternalInput").ap()
    bout = nc.dram_tensor("bout", [C], F32, kind="ExternalInput").ap()
    y = nc.dram_tensor("y", [T, C], F32, kind="ExternalOutput").ap()

    xT3 = xT.rearrange("(o p) t -> p o t", p=128)  # [128, 6, 4096]
    y3 = y.rearrange("(n p) e -> p n e", p=128)  # [128, 32, 768]

    with tile.TileContext(nc) as tc, ExitStack() as ctx:
        sb = ctx.enter_context(tc.tile_pool(name="persist", bufs=1))

        # --- weights / biases / x (one-shot loads) ---
        x16 = sb.tile([128, 6, T], F16)
        nc.sync.dma_start(x16[:, 0:3, :], xT3[:, 0:3, :])
        nc.gpsimd.dma_start(x16[:, 3:6, :], xT3[:, 3:6, :])
        wqk_sb = sb.tile([128, 6, 384], F16)
        nc.sync.dma_start(wqk_sb[:], wqk.rearrange("(o p) c -> p o c", p=128))
        wv_sb = sb.tile([128, 6, 192], F16)
        nc.sync.dma_start(wv_sb[:], wv.rearrange("(o p) c -> p o c", p=128))
        wout_sb = sb.tile([64, 3, C], F16)
        nc.sync.dma_start(wout_sb[:], wout.rearrange("(h p) e -> p h e", p=64))
        bqk_sb = sb.tile([128, 3], F32)
        nc.sync.dma_start(bqk_sb[:], bqk.rearrange("(o p) -> p o", p=128))
        bv_bc = sb.tile([128, 192], F32)
        nc.sync.dma_start(bv_bc[:], bv[None, :].to_broadcast((128, 192)))
        bout_bc = sb.tile([128, C], F32)
        nc.sync.dma_start(bout_bc[:], bout[None, :].to_broadcast((128, C)))

        # --- persistent activations ---
        # qkT slots: o0=[q0|q1] o1=[k0|k1] o2=[q2|k2] o3=[k2|q2]
        qkT = sb.tile([128, 4, T], F16)
        # V per tk-block per head, token-major, with ones col at [.., 64]
        vaug = sb.tile([128, NTK, HPC, 65], F16)
        nc.vector.memset(vaug[:, :, :, 64:65], 1.0)

        # --- prologue: qkv projections ---
        with (
            tc.tile_pool(name="psQK", bufs=2, space="PSUM") as psQK,
            tc.tile_pool(name="psV", bufs=2, space="PSUM") as psV,
        ):
            for tb in range(NTQ):
                for cb in range(3):
                    ps = psQK.tile([128, GQ], F32, tag="qk")
                    for d in range(6):
                        nc.tensor.matmul(
                            ps[:],
                            (wqk_sb[:, d, ts(cb, 128)]),
                            (x16[:, d, ts(tb, GQ)]),
                            start=(d == 0),
                            stop=(d == 5),
                        )
                    # bias-add + f16 copy on ACT (idle in prologue)
                    nc.scalar.activation(
                        qkT[:, cb, ts(tb, GQ)],
                        ps[:],
                        mybir.ActivationFunctionType.Identity,
                        bias=bqk_sb[:, cb : cb + 1],
                    )
                # o3 = swap halves of o2 (k2|q2)
                nc.gpsimd.dma_start(
                    qkT[0:64, 3, ts(tb, GQ)], qkT[64:128, 2, ts(tb, GQ)]
                )
                nc.gpsimd.dma_start(
                    qkT[64:128, 3, ts(tb, GQ)], qkT[0:64, 2, ts(tb, GQ)]
                )

                # v-projection (f16), 128-token sub-blocks
                for sub in range(GQ // 128):
                    g = tb * (GQ // 128) + sub
                    psv = psV.tile([128, 192], F32, tag="v")
                    for d in range(6):
                        nc.tensor.matmul(
                            psv[:],
                            (x16[:, d, ts(g, 128)]),
                            (wv_sb[:, d, :]),
                            start=(d == 0),
                            stop=(d == 5),
                        )
                    nc.vector.tensor_tensor(
                        vaug[:, g, :, 0:64],
                        psv[:].rearrange("p (h d) -> p h d", d=64),
                        bv_bc[:].rearrange("p (h d) -> p h d", d=64),
                        mybir.AluOpType.add,
                    )

        # --- attention + output projection ---
        with (
            tc.tile_pool(name="psST", bufs=2, space="PSUM") as psST,
            tc.tile_pool(name="psOT", bufs=3, space="PSUM") as psOT,
            tc.tile_pool(name="psY", bufs=1, space="PSUM") as psY,
            tc.tile_pool(name="estA", bufs=3) as estA,
            tc.tile_pool(name="estD", bufs=3) as estD,
            tc.tile_pool(name="otp", bufs=2) as otp,
            tc.tile_pool(name="smallp", bufs=3) as smallp,
            tc.tile_pool(name="yp", bufs=2) as yp,
            tc.tile_pool(name="dramp", bufs=3, space="DRAM") as dramp,
        ):

            def expst(st, idx):
                """exp(st*0.125): 3/5 exact-ACT, 2/5 Schraudolph-DVE."""
                if idx % 5 not in (1, 3):
                    est = estA.tile([128, 2, GQ], F16, tag="estA")
                    nc.scalar.activation(
                        est[:], st[:], mybir.ActivationFunctionType.Exp, scale=0.125
                    )
                    return est
                esti = estD.tile([128, 2, GQ], I16, tag="estD")
                nc.vector.tensor_scalar(
                    esti[:],
                    st[:],
                    K_SCH,
                    B_SCH,
                    mybir.AluOpType.mult,
                    mybir.AluOpType.add,
                )
                return esti.bitcast(F16)

            def normalize(ps_ot, ot_dst):
                # recip_approx_fast's bit-trick seed misbehaves on PSUM
                # reads - stage the denominator row through SBUF first
                sd = smallp.tile([1, GQ], F32, tag="sd")
                nc.vector.tensor_copy(sd[:], ps_ot[64:65, :])
                rc = smallp.tile([1, GQ], F32, tag="rc")
                nc.vector.reciprocal_approx_fast(rc[:], sd[:])
                dn = dramp.tile([GQ], F32, tag="dn")
                nc.gpsimd.dma_start(dn[:], rc[:])
                rb = smallp.tile([64, GQ], F32, tag="rb")
                nc.gpsimd.dma_start(rb[:], dn[None, :].to_broadcast((64, GQ)))
                nc.vector.tensor_tensor(
                    ot_dst, ps_ot[0:64, :], rb[:], mybir.AluOpType.mult
                )

            def out_proj(tq, ot_tile):
                for tsub in range(GQ // 128):
                    y_sb = yp.tile([128, C], F32, tag="y_sb")
                    for nh in range(2):
                        py = psY.tile([128, 384], F32, tag="y")
                        for h in range(HPC):
                            nc.tensor.matmul(
                                py[:],
                                (ot_tile[:, h, ts(tsub, 128)]),
                                (wout_sb[:, h, ts(nh, 384)]),
                                start=(h == 0),
                                stop=(h == HPC - 1),
                            )
                        nc.vector.tensor_tensor(
                            y_sb[:, ts(nh, 384)],
                            py[:],
                            bout_bc[:, ts(nh, 384)],
                            mybir.AluOpType.add,
                        )
                    nc.sync.dma_start(y3[:, tq * (GQ // 128) + tsub, :], y_sb[:])

            prev = None  # (tq, ot_tile) pending out-projection
            for tq in range(NTQ):
                ot_tile = otp.tile([64, HPC, GQ], F16, tag="ot_sb")

                # -- phase A: heads 0,1 (paired on PE rows lo/hi) --
                # software-pipelined: ST/exp issued 2 iterations ahead of AV
                ps_ot0 = psOT.tile([65, GQ], F32, tag="ot")
                ps_ot1 = psOT.tile([65, GQ], F32, tag="ot")
                ests = {}
                for i in range(NTK + 2):
                    if i < NTK:
                        st = psST.tile([128, 2, GQ], F32, tag="st")
                        nc.tensor.matmul(
                            st[:, 0, :],
                            (qkT[0:64, 1, ts(i, 128)]),
                            (qkT[0:64, 0, ts(tq, GQ)]),
                            start=True,
                            stop=True,
                        )
                        nc.tensor.matmul(
                            st[:, 1, :],
                            (qkT[64:128, 1, ts(i, 128)]),
                            (qkT[64:128, 0, ts(tq, GQ)]),
                            start=True,
                            stop=True,
                        )
                        ests[i] = expst(st, i)
                    if i >= 2:
                        g = i - 2
                        est = ests.pop(g)
                        nc.tensor.matmul(
                            ps_ot0[:],
                            (vaug[:, g, 0, :]),
                            (est[:, 0, :]),
                            start=(g == 0),
                            stop=(g == NTK - 1),
                        )
                        nc.tensor.matmul(
                            ps_ot1[:],
                            (vaug[:, g, 1, :]),
                            (est[:, 1, :]),
                            start=(g == 0),
                            stop=(g == NTK - 1),
                        )
                normalize(ps_ot0, ot_tile[:, 0, :])
                normalize(ps_ot1, ot_tile[:, 1, :])

                # -- phase B: head 2 (paired across even/odd tk-blocks) --
                ps_ot2 = psOT.tile([65, GQ], F32, tag="ot")
                for i in range(NTK // 2 + 2):
                    if i < NTK // 2:
                        ge, go = 2 * i, 2 * i + 1
                        st = psST.tile([128, 2, GQ], F32, tag="st")
                        nc.tensor.matmul(
                            st[:, 0, :],
                            (qkT[0:64, 3, ts(ge, 128)]),
                            (qkT[0:64, 2, ts(tq, GQ)]),
                            start=True,
                            stop=True,
                        )
                        nc.tensor.matmul(
                            st[:, 1, :],
                            (qkT[64:128, 2, ts(go, 128)]),
                            (qkT[64:128, 3, ts(tq, GQ)]),
                            start=True,
                            stop=True,
                        )
                        ests[i] = expst(st, i)
                    if i >= 2:
                        g2 = i - 2
                        ge, go = 2 * g2, 2 * g2 + 1
                        est = ests.pop(g2)
                        nc.tensor.matmul(
                            ps_ot2[:],
                            (vaug[:, ge, 2, :]),
                            (est[:, 0, :]),
                            start=(g2 == 0),
                            stop=False,
                        )
                        nc.tensor.matmul(
                            ps_ot2[:],
                            (vaug[:, go, 2, :]),
                            (est[:, 1, :]),
                            start=False,
                            stop=(g2 == NTK // 2 - 1),
                        )
                normalize(ps_ot2, ot_tile[:, 2, :])
                # out-projection pipelined one tq behind (its ot is long ready)
                if prev is not None:
                    out_proj(*prev)
                prev = (tq, ot_tile)
            out_proj(*prev)

    nc.compile()
    return nc


_PROGRAM = None


def _get_program():
    global _PROGRAM
    if _PROGRAM is None:
        _PROGRAM = _build_program()
    return _PROGRAM


def _make_in_maps(x, W_qkv, b_qkv, W_out, b_out):
    x = np.asarray(x, dtype=np.float32)
    W_qkv = np.asarray(W_qkv, dtype=np.float32)
    b_qkv = np.asarray(b_qkv, dtype=np.float32)
    W_out = np.asarray(W_out, dtype=np.float32)
    b_out = np.asarray(b_out, dtype=np.float32)

    xT_b = [np.ascontiguousarray(x[b].T).astype(np.float16) for b in range(B)]
    in_maps = []
    for c in range(N_CORES):
        b, g = divmod(c, 4)
        h0 = HPC * g

        def qcol(h):
            return slice(h * DK, (h + 1) * DK)

        def kcol(h):
            return slice(C + h * DK, C + (h + 1) * DK)

        wqk_c = np.concatenate(
            [
                W_qkv[:, qcol(h0)],
                W_qkv[:, qcol(h0 + 1)],
                W_qkv[:, kcol(h0)],
                W_qkv[:, kcol(h0 + 1)],
                W_qkv[:, qcol(h0 + 2)],
                W_qkv[:, kcol(h0 + 2)],
            ],
            axis=1,
        )
        bqk_c = np.concatenate(
            [
                b_qkv[qcol(h0)],
                b_qkv[qcol(h0 + 1)],
                b_qkv[kcol(h0)],
                b_qkv[kcol(h0 + 1)],
                b_qkv[qcol(h0 + 2)],
                b_qkv[kcol(h0 + 2)],
            ]
        )
        vs = slice(2 * C + h0 * DK, 2 * C + (h0 + HPC) * DK)
        in_maps.append(
            {
                "xT": xT_b[b],
                "wqk": np.ascontiguousarray(wqk_c).astype(np.float16),
                "bqk": np.ascontiguousarray(bqk_c),
                "wv": np.ascontiguousarray(W_qkv[:, vs]).astype(np.float16),
                "bv": np.ascontiguousarray(b_qkv[vs]),
                "wout": np.ascontiguousarray(
                    W_out[h0 * DK : (h0 + HPC) * DK, :]
                ).astype(np.float16),
                "bout": (b_out if g == 0 else np.zeros_like(b_out)).copy(),
            }
        )
    return in_maps


def _assemble(results):
    out = np.zeros((B, T, C), dtype=np.float32)
    for c in range(N_CORES):
        out[c // 4] += results[c]["y"]
    return out


def kernel_run(inputs, trace=False):
    """Returns (full_output [B,T,C] fp32, exec_time_ns or None)."""
    nc = _get_program()
    in_maps = _make_in_maps(**inputs)
    res = run_bass_kernel_spmd(
        nc, in_maps, core_ids=list(range(N_CORES)), trace=trace
    )
    return _assemble(res.results), res.exec_time_ns


def kernel(**inputs):
    out, _ = kernel_run(inputs)
    return out


# revision 7
# speedup vs baseline: 1.7006x; 1.0113x over previous
"""Multi-head attention (B=2, T=4096, C=768, H=12, Dk=64) on 8 trn2 NeuronCores.

Sharding: core c -> batch b = c//4, head-group g = c%4 (3 heads each).
Megatron-style: each core computes qkv projection for its 3 heads, full
attention for those heads, and a row-parallel partial of the output
projection. Host sums the 4 partials per batch (+ bias, folded into the
g==0 core's partial on device).

v4:
  - exp split ~0.35 to DVE (Schraudolph f16-bit, mean-zero offset), with
    DVE's share scheduled mid-phase so it never collides with the
    normalize/y-copy DVE bursts at tq boundaries.
  - out-projection packed into two [128,128]-stationary matmuls (h0|h1
    stacked via an SBUF partition-move DMA; h2 zero-padded) - the 64-row
    stationary form can't use the fast weight-load path.
  - softmax denominator row copied out of PSUM on ACT; reciprocal via
    reciprocal_approx_fast (SBUF input only - PSUM reads misbehave).
  - x & W_qk f16; x loaded once in 4 chunks on 2 DMA queues.
  - software-pipelined attention (ST 2 ahead of AV) + out-projection one
    tq behind; normalize roundtrips and o3-swaps on the gpsimd queue.
"""

import os
import sys
from contextlib import ExitStack

import numpy as np

for _p in ("/opt/trn_rl_repo", "/root/.axon_site/_ro/trn_rl_repo"):
    if os.path.isdir(_p) and _p not in sys.path:
        sys.path.append(_p)

import concourse.bass as bass
import concourse.mybir as mybir
import concourse.tile as tile
from concourse import bacc
from concourse.bass import ts
from concourse.bass_utils import run_bass_kernel_spmd

F32 = mybir.dt.float32
F16 = mybir.dt.float16
I16 = mybir.dt.int16

B, T, C = 2, 4096, 768
H, DK = 12, 64
N_CORES = 8
HPC = 3  # heads per core
GQ = 512  # q-block (matmul free dim)
NTQ = T // GQ  # 8 q-blocks
NTK = T // 128  # 32 tk-blocks

# Schraudolph f16-bit exp: exp(s/8) ~ bits_f16(round(s*K + B)), mean-zero
# offset so the approx blocks carry no systematic bias vs the exact blocks
# sharing the same softmax denominator.
K_SCH = 1024.0 * float(np.log2(np.e)) / 8.0
B_SCH = 15308.5
# which ST blocks take the DVE-approx exp (mid-phase only: the DVE is busy
# with normalize + y copies around tq boundaries)
DVE_A = {8, 10, 12, 14, 16, 18, 20, 22, 24, 26, 28}
DVE_B = {4, 6, 8, 10, 12, 14}


def _build_program():
    nc = bacc.Bacc("TRN2", target_bir_lowering=False, debug=False)

    xT = nc.dram_tensor("xT", [C, T], F16, kind="ExternalInput").ap()
    wqk = nc.dram_tensor("wqk", [C, 384], F16, kind="ExternalInput").ap()
    bqk = nc.dram_tensor("bqk", [384], F32, kind="ExternalInput").ap()
    wv = nc.dram_tensor("wv", [C, 192], F16, kind="ExternalInput").ap()
    bv = nc.dram_tensor("bv", [192], F32, kind="ExternalInput").ap()
    wout = nc.dram_tensor("wout", [192, C], F16, kind="ExternalInput").ap()
    bout = nc.dram_tensor("bout", [C], F32, kind="ExternalInput").ap()
    y = nc.dram_tensor("y", [T, C], F32, kind="ExternalOutput").ap()

    xT3 = xT.rearrange("(o p) t -> p o t", p=128)  # [128, 6, 4096]
    y3 = y.rearrange("(n p) e -> p n e", p=128)  # [128, 32, 768]

    with tile.TileContext(nc) as tc, ExitStack() as ctx:
        sb = ctx.enter_context(tc.tile_pool(name="persist", bufs=1))

        # --- weights / biases first (small), then x in 4 chunks / 2 queues ---
        wqk_sb = sb.tile([128, 6, 384], F16)
        nc.sync.dma_start(wqk_sb[:], wqk.rearrange("(o p) c -> p o c", p=128))
        wv_sb = sb.tile([128, 6, 192], F16)
        nc.sync.dma_start(wv_sb[:], wv.rearrange("(o p) c -> p o c", p=128))
        # out-proj weights: slot0 = [h0;h1] rows, slot1 = [h2; zeros]
        wout_sb = sb.tile([128, 2, C], F16)
        nc.gpsimd.dma_start(wout_sb[:, 0, :], wout[0:128, :])
        nc.gpsimd.dma_start(wout_sb[0:64, 1, :], wout[128:192, :])
        nc.vector.memset(wout_sb[64:128, 1, :], 0.0)
        bqk_sb = sb.tile([128, 3], F32)
        nc.sync.dma_start(bqk_sb[:], bqk.rearrange("(o p) -> p o", p=128))
        bv_bc = sb.tile([128, 192], F32)
        nc.sync.dma_start(bv_bc[:], bv[None, :].to_broadcast((128, 192)))
        bout_bc = sb.tile([128, C], F32)
        nc.sync.dma_start(bout_bc[:], bout[None, :].to_broadcast((128, C)))

        x16 = sb.tile([128, 6, T], F16)
        for cchunk in range(4):
            eng = nc.sync if cchunk % 2 == 0 else nc.gpsimd
            eng.dma_start(
                x16[:, :, ts(cchunk, T // 4)], xT3[:, :, ts(cchunk, T // 4)]
            )

        # --- persistent activations ---
        # qkT slots: o0=[q0|q1] o1=[k0|k1] o2=[q2|k2] o3=[k2|q2]
        qkT = sb.tile([128, 4, T], F16)
        # V per tk-block per head, token-major, with ones col at [.., 64]
        vaug = sb.tile([128, NTK, HPC, 65], F16)
        nc.vector.memset(vaug[:, :, :, 64:65], 1.0)
        # normalized attention outputs, double-buffered across tq:
        # slot0 = [ot_h0 (p 0:64); ot_h1 (p 64:128)], slot1 = [ot_h2; zeros]
        ot_bufs = []
        for bi in range(2):
            otb = sb.tile([128, 2, GQ], F16, name=f"otb{bi}")
            nc.vector.memset(otb[64:128, 1, :], 0.0)
            ot_bufs.append(otb)

        # --- prologue: qkv projections ---
        with (
            tc.tile_pool(name="psQK", bufs=2, space="PSUM") as psQK,
            tc.tile_pool(name="psV", bufs=2, space="PSUM") as psV,
        ):
            for tb in range(NTQ):
                for cb in range(3):
                    ps = psQK.tile([128, GQ], F32, tag="qk")
                    for d in range(6):
                        nc.tensor.matmul(
                            ps[:],
                            (wqk_sb[:, d, ts(cb, 128)]),
                            (x16[:, d, ts(tb, GQ)]),
                            start=(d == 0),
                            stop=(d == 5),
                        )
                    # bias-add + f16 copy on ACT (idle in prologue)
                    nc.scalar.activation(
                        qkT[:, cb, ts(tb, GQ)],
                        ps[:],
                        mybir.ActivationFunctionType.Identity,
                        bias=bqk_sb[:, cb : cb + 1],
                    )
                # o3 = swap halves of o2 (k2|q2)
                nc.gpsimd.dma_start(
                    qkT[0:64, 3, ts(tb, GQ)], qkT[64:128, 2, ts(tb, GQ)]
                )
                nc.gpsimd.dma_start(
                    qkT[64:128, 3, ts(tb, GQ)], qkT[0:64, 2, ts(tb, GQ)]
                )

                # v-projection (f16), 128-token sub-blocks
                for sub in range(GQ // 128):
                    g = tb * (GQ // 128) + sub
                    psv = psV.tile([128, 192], F32, tag="v")
                    for d in range(6):
                        nc.tensor.matmul(
                            psv[:],
                            (x16[:, d, ts(g, 128)]),
                            (wv_sb[:, d, :]),
                            start=(d == 0),
                            stop=(d == 5),
                        )
                    nc.vector.tensor_tensor(
                        vaug[:, g, :, 0:64],
                        psv[:].rearrange("p (h d) -> p h d", d=64),
                        bv_bc[:].rearrange("p (h d) -> p h d", d=64),
                        mybir.AluOpType.add,
                    )

        # --- attention + output projection ---
        with (
            tc.tile_pool(name="psST", bufs=2, space="PSUM") as psST,
            tc.tile_pool(name="psOT", bufs=3, space="PSUM") as psOT,
            tc.tile_pool(name="psY", bufs=1, space="PSUM") as psY,
            tc.tile_pool(name="estA", bufs=3) as estA,
            tc.tile_pool(name="estD", bufs=3) as estD,
            tc.tile_pool(name="smallp", bufs=3) as smallp,
            tc.tile_pool(name="yp", bufs=2) as yp,
            tc.tile_pool(name="dramp", bufs=3, space="DRAM") as dramp,
        ):

            def expst(st, idx, dve_set):
                """exp(st*0.125): exact-ACT or Schraudolph-DVE."""
                if idx not in dve_set:
                    est = estA.tile([128, 2, GQ], F16, tag="estA")
                    nc.scalar.activation(
                        est[:], st[:], mybir.ActivationFunctionType.Exp, scale=0.125
                    )
                    return est
                esti = estD.tile([128, 2, GQ], I16, tag="estD")
                nc.vector.tensor_scalar(
                    esti[:],
                    st[:],
                    K_SCH,
                    B_SCH,
                    mybir.AluOpType.mult,
                    mybir.AluOpType.add,
                )
                return esti.bitcast(F16)

            def normalize(ps_ot, ot_dst):
                # ACT stages the denominator row out of PSUM (DVE is the
                # congested engine at tq boundaries); recip_approx_fast
                # requires SBUF input.
                sd = smallp.tile([1, GQ], F32, tag="sd")
                nc.scalar.copy(sd[:], ps_ot[64:65, :])
                rc = smallp.tile([1, GQ], F32, tag="rc")
                nc.vector.reciprocal_approx_fast(rc[:], sd[:])
                dn = dramp.tile([GQ], F32, tag="dn")
                nc.gpsimd.dma_start(dn[:], rc[:])
                rb = smallp.tile([64, GQ], F32, tag="rb")
                nc.gpsimd.dma_start(rb[:], dn[None, :].to_broadcast((64, GQ)))
                nc.vector.tensor_tensor(
                    ot_dst, ps_ot[0:64, :], rb[:], mybir.AluOpType.mult
                )

            def out_proj(tq, ot_tile):
                for tsub in range(GQ // 128):
                    y_sb = yp.tile([128, C], F32, tag="y_sb")
                    for nh in range(2):
                        py = psY.tile([128, 384], F32, tag="y")
                        for sl in range(2):
                            nc.tensor.matmul(
                                py[:],
                                (ot_tile[:, sl, ts(tsub, 128)]),
                                (wout_sb[:, sl, ts(nh, 384)]),
                                start=(sl == 0),
                                stop=(sl == 1),
                            )
                        nc.vector.tensor_tensor(
                            y_sb[:, ts(nh, 384)],
                            py[:],
                            bout_bc[:, ts(nh, 384)],
                            mybir.AluOpType.add,
                        )
                    nc.sync.dma_start(y3[:, tq * (GQ // 128) + tsub, :], y_sb[:])

            prev = None  # (tq, ot_tile) pending out-projection
            for tq in range(NTQ):
                ot_tile = ot_bufs[tq % 2]

                # -- phase A: heads 0,1 (paired on PE rows lo/hi) --
                # software-pipelined: ST/exp issued 2 iterations ahead of AV
                ps_ot0 = psOT.tile([65, GQ], F32, tag="ot")
                ps_ot1 = psOT.tile([65, GQ], F32, tag="ot")
                ests = {}
                for i in range(NTK + 2):
                    if i < NTK:
                        st = psST.tile([128, 2, GQ], F32, tag="st")
                        nc.tensor.matmul(
                            st[:, 0, :],
                            (qkT[0:64, 1, ts(i, 128)]),
                            (qkT[0:64, 0, ts(tq, GQ)]),
                            start=True,
                            stop=True,
                        )
                        nc.tensor.matmul(
                            st[:, 1, :],
                            (qkT[64:128, 1, ts(i, 128)]),
                            (qkT[64:128, 0, ts(tq, GQ)]),
                            start=True,
                            stop=True,
                        )
                        ests[i] = expst(st, i, DVE_A)
                    if i >= 2:
                        g = i - 2
                        est = ests.pop(g)
                        nc.tensor.matmul(
                            ps_ot0[:],
                            (vaug[:, g, 0, :]),
                            (est[:, 0, :]),
                            start=(g == 0),
                            stop=(g == NTK - 1),
                        )
                        nc.tensor.matmul(
                            ps_ot1[:],
                            (vaug[:, g, 1, :]),
                            (est[:, 1, :]),
                            start=(g == 0),
                            stop=(g == NTK - 1),
                        )
                normalize(ps_ot0, ot_tile[0:64, 0, :])
                # h1 -> staging tile, then partition-move to ot rows 64:128
                ot1s = smallp.tile([64, GQ], F16, tag="ot1s")
                normalize(ps_ot1, ot1s[:])
                nc.gpsimd.dma_start(ot_tile[64:128, 0, :], ot1s[:])

                # -- phase B: head 2 (paired across even/odd tk-blocks) --
                ps_ot2 = psOT.tile([65, GQ], F32, tag="ot")
                for i in range(NTK // 2 + 2):
                    if i < NTK // 2:
                        ge, go = 2 * i, 2 * i + 1
                        st = psST.tile([128, 2, GQ], F32, tag="st")
                        nc.tensor.matmul(
                            st[:, 0, :],
                            (qkT[0:64, 3, ts(ge, 128)]),
                            (qkT[0:64, 2, ts(tq, GQ)]),
                            start=True,
                            stop=True,
                        )
                        nc.tensor.matmul(
                            st[:, 1, :],
                            (qkT[64:128, 2, ts(go, 128)]),
                            (qkT[64:128, 3, ts(tq, GQ)]),
                            start=True,
                            stop=True,
                        )
                        ests[i] = expst(st, i, DVE_B)
                    if i >= 2:
                        g2 = i - 2
                        ge, go = 2 * g2, 2 * g2 + 1
                        est = ests.pop(g2)
                        nc.tensor.matmul(
                            ps_ot2[:],
                            (vaug[:, ge, 2, :]),
                            (est[:, 0, :]),
                            start=(g2 == 0),
                            stop=False,
                        )
                        nc.tensor.matmul(
                            ps_ot2[:],
                            (vaug[:, go, 2, :]),
                            (est[:, 1, :]),
                            start=False,
                            stop=(g2 == NTK // 2 - 1),
                        )
                normalize(ps_ot2, ot_tile[0:64, 1, :])
                # out-projection pipelined one tq behind (its ot is long ready)
                if prev is not None:
                    out_proj(*prev)
                prev = (tq, ot_tile)
            out_proj(*prev)

    nc.compile()
    return nc


_PROGRAM = None


def _get_program():
    global _PROGRAM
    if _PROGRAM is None:
        _PROGRAM = _build_program()
    return _PROGRAM


def _make_in_maps(x, W_qkv, b_qkv, W_out, b_out):
    x = np.asarray(x, dtype=np.float32)
    W_qkv = np.asarray(W_qkv, dtype=np.float32)
    b_qkv = np.asarray(b_qkv, dtype=np.float32)
    W_out = np.asarray(W_out, dtype=np.float32)
    b_out = np.asarray(b_out, dtype=np.float32)

    xT_b = [np.ascontiguousarray(x[b].T).astype(np.float16) for b in range(B)]
    in_maps = []
    for c in range(N_CORES):
        b, g = divmod(c, 4)
        h0 = HPC * g

        def qcol(h):
            return slice(h * DK, (h + 1) * DK)

        def kcol(h):
            return slice(C + h * DK, C + (h + 1) * DK)

        wqk_c = np.concatenate(
            [
                W_qkv[:, qcol(h0)],
                W_qkv[:, qcol(h0 + 1)],
                W_qkv[:, kcol(h0)],
                W_qkv[:, kcol(h0 + 1)],
                W_qkv[:, qcol(h0 + 2)],
                W_qkv[:, kcol(h0 + 2)],
            ],
            axis=1,
        )
        bqk_c = np.concatenate(
            [
                b_qkv[qcol(h0)],
                b_qkv[qcol(h0 + 1)],
                b_qkv[kcol(h0)],
                b_qkv[kcol(h0 + 1)],
                b_qkv[qcol(h0 + 2)],
                b_qkv[kcol(h0 + 2)],
            ]
        )
        vs = slice(2 * C + h0 * DK, 2 * C + (h0 + HPC) * DK)
        in_maps.append(
            {
                "xT": xT_b[b],
                "wqk": np.ascontiguousarray(wqk_c).astype(np.float16),
                "bqk": np.ascontiguousarray(bqk_c),
                "wv": np.ascontiguousarray(W_qkv[:, vs]).astype(np.float16),
                "bv": np.ascontiguousarray(b_qkv[vs]),
                "wout": np.ascontiguousarray(
                    W_out[h0 * DK : (h0 + HPC) * DK, :]
                ).astype(np.float16),
                "bout": (b_out if g == 0 else np.zeros_like(b_out)).copy(),
            }
        )
    return in_maps


def _assemble(results):
    out = np.zeros((B, T, C), dtype=np.float32)
    for c in range(N_CORES):
        out[c // 4] += results[c]["y"]
    return out


def kernel_run(inputs, trace=False):
    """Returns (full_output [B,T,C] fp32, exec_time_ns or None)."""
    nc = _get_program()
    in_maps = _make_in_maps(**inputs)
    res = run_bass_kernel_spmd(
        nc, in_maps, core_ids=list(range(N_CORES)), trace=trace
    )
    return _assemble(res.results), res.exec_time_ns


def kernel(**inputs):
    out, _ = kernel_run(inputs)
    return out


# revision 22
# speedup vs baseline: 1.7420x; 1.0244x over previous
"""Multi-head attention (B=2, T=4096, C=768, H=12, Dk=64) on 8 trn2 NeuronCores.

Sharding: core c -> batch b = c//4, head-group g = c%4 (3 heads each).
Megatron-style: each core computes qkv projection for its 3 heads, full
attention for those heads, and a row-parallel partial of the output
projection. Host sums the 4 partials per batch (+ bias, folded into the
g==0 core's partial on device).

v4:
  - exp split ~0.35 to DVE (Schraudolph f16-bit, mean-zero offset), with
    DVE's share scheduled mid-phase so it never collides with the
    normalize/y-copy DVE bursts at tq boundaries.
  - out-projection packed into two [128,128]-stationary matmuls (h0|h1
    stacked via an SBUF partition-move DMA; h2 zero-padded) - the 64-row
    stationary form can't use the fast weight-load path.
  - softmax denominator row copied out of PSUM on ACT; reciprocal via
    reciprocal_approx_fast (SBUF input only - PSUM reads misbehave).
  - x & W_qk f16; x loaded once in 4 chunks on 2 DMA queues.
  - software-pipelined attention (ST 2 ahead of AV) + out-projection one
    tq behind; normalize roundtrips and o3-swaps on the gpsimd queue.
"""

import os
import sys
from contextlib import ExitStack

import numpy as np

for _p in ("/opt/trn_rl_repo", "/root/.axon_site/_ro/trn_rl_repo"):
    if os.path.isdir(_p) and _p not in sys.path:
        sys.path.append(_p)

import concourse.bass as bass
import concourse.mybir as mybir
import concourse.tile as tile
from concourse import bacc
from concourse.bass import ts
from concourse.bass_utils import run_bass_kernel_spmd

F32 = mybir.dt.float32
F16 = mybir.dt.float16
I16 = mybir.dt.int16

B, T, C = 2, 4096, 768
H, DK = 12, 64
N_CORES = 8
HPC = 3  # heads per core
GQ = 512  # q-block (matmul free dim)
NTQ = T // GQ  # 8 q-blocks
NTK = T // 128  # 32 tk-blocks

# Schraudolph f16-bit exp: exp(s/8) ~ bits_f16(round(s*K + B)), mean-zero
# offset so the approx blocks carry no systematic bias vs the exact blocks
# sharing the same softmax denominator.
K_SCH = 1024.0 * float(np.log2(np.e)) / 8.0
B_SCH = 15308.5
# which ST blocks take the DVE-approx exp (mid-phase only: the DVE is busy
# with normalize + y copies around tq boundaries)
DVE_A = {8, 10, 12, 14, 16, 18, 20, 22, 24, 26, 28}
DVE_B = {4, 6, 8, 10, 12, 14}


def _build_program():
    nc = bacc.Bacc("TRN2", target_bir_lowering=False, debug=False)

    # xT/wqk/wv/bqk come host-pre-arranged partition-major: loads are one
    # contiguous run per partition (128 DMA descriptors, not 768)
    xT = nc.dram_tensor("xT", [128, 6, T], F16, kind="ExternalInput").ap()
    wqk = nc.dram_tensor("wqk", [128, 6, 384], F16, kind="ExternalInput").ap()
    bqk = nc.dram_tensor("bqk", [128, 3], F32, kind="ExternalInput").ap()
    wv = nc.dram_tensor("wv", [128, 6, 192], F16, kind="ExternalInput").ap()
    bv = nc.dram_tensor("bv", [192], F32, kind="ExternalInput").ap()
    wout = nc.dram_tensor("wout", [192, C], F16, kind="ExternalInput").ap()
    bout = nc.dram_tensor("bout", [C], F32, kind="ExternalInput").ap()
    y = nc.dram_tensor("y", [T, C], F32, kind="ExternalOutput").ap()

    y3 = y.rearrange("(n p) e -> p n e", p=128)  # [128, 32, 768]

    with tile.TileContext(nc) as tc, ExitStack() as ctx:
        sb = ctx.enter_context(tc.tile_pool(name="persist", bufs=1))

        # --- weights / biases first (small), then x in 4 chunks / 2 queues ---
        wqk_sb = sb.tile([128, 6, 384], F16)
        nc.sync.dma_start(wqk_sb[:], wqk)
        wv_sb = sb.tile([128, 6, 192], F16)
        nc.sync.dma_start(wv_sb[:], wv)
        # out-proj weights: one [128,128]-stationary slot per head, rows
        # 64:128 zero-padded (written once; [64,x] stationaries can't use
        # the fast weight-load path)
        wout_sb = sb.tile([128, 3, C], F16)
        for h in range(HPC):
            nc.gpsimd.dma_start(wout_sb[0:64, h, :], wout[ts(h, 64), :])
        nc.vector.memset(wout_sb[64:128, :, :], 0.0)
        bqk_sb = sb.tile([128, 3], F32)
        nc.sync.dma_start(bqk_sb[:], bqk)
        bv_bc = sb.tile([128, 192], F32)
        nc.sync.dma_start(bv_bc[:], bv[None, :].to_broadcast((128, 192)))
        bout_bc = sb.tile([128, C], F32)
        nc.sync.dma_start(bout_bc[:], bout[None, :].to_broadcast((128, C)))

        x16 = sb.tile([128, 6, T], F16)
        nc.sync.dma_start(x16[:, 0:3, :], xT[:, 0:3, :])
        nc.gpsimd.dma_start(x16[:, 3:6, :], xT[:, 3:6, :])

        # --- persistent activations ---
        # qkT slots: o0=[q0|q1] o1=[k0|k1] o2=[q2|k2] o3=[k2|q2]
        qkT = sb.tile([128, 4, T], F16)
        # V per tk-block per head, token-major, with ones col at [.., 64]
        vaug = sb.tile([128, NTK, HPC, 65], F16)
        nc.vector.memset(vaug[:, :, :, 64:65], 1.0)
        # normalized attention outputs, double-buffered across tq; rows
        # 64:128 zero-padded once to match the padded wout stationaries
        ot_bufs = []
        for bi in range(2):
            otb = sb.tile([128, HPC, GQ], F16, name=f"otb{bi}")
            nc.vector.memset(otb[64:128, :, :], 0.0)
            ot_bufs.append(otb)

        # --- prologue: qkv projections ---
        with (
            tc.tile_pool(name="psQK", bufs=2, space="PSUM") as psQK,
            tc.tile_pool(name="psV", bufs=2, space="PSUM") as psV,
        ):
            for tb in range(NTQ):
                for cb in range(3):
                    ps = psQK.tile([128, GQ], F32, tag="qk")
                    for d in range(6):
                        nc.tensor.matmul(
                            ps[:],
                            (wqk_sb[:, d, ts(cb, 128)]),
                            (x16[:, d, ts(tb, GQ)]),
                            start=(d == 0),
                            stop=(d == 5),
                        )
                    # bias-add + f16 copy on ACT (idle in prologue)
                    nc.scalar.activation(
                        qkT[:, cb, ts(tb, GQ)],
                        ps[:],
                        mybir.ActivationFunctionType.Identity,
                        bias=bqk_sb[:, cb : cb + 1],
                    )
                # o3 = swap halves of o2 (k2|q2)
                nc.gpsimd.dma_start(
                    qkT[0:64, 3, ts(tb, GQ)], qkT[64:128, 2, ts(tb, GQ)]
                )
                nc.gpsimd.dma_start(
                    qkT[64:128, 3, ts(tb, GQ)], qkT[0:64, 2, ts(tb, GQ)]
                )

                # v-projection (f16), 128-token sub-blocks
                for sub in range(GQ // 128):
                    g = tb * (GQ // 128) + sub
                    psv = psV.tile([128, 192], F32, tag="v")
                    for d in range(6):
                        nc.tensor.matmul(
                            psv[:],
                            (x16[:, d, ts(g, 128)]),
                            (wv_sb[:, d, :]),
                            start=(d == 0),
                            stop=(d == 5),
                        )
                    nc.vector.tensor_tensor(
                        vaug[:, g, :, 0:64],
                        psv[:].rearrange("p (h d) -> p h d", d=64),
                        bv_bc[:].rearrange("p (h d) -> p h d", d=64),
                        mybir.AluOpType.add,
                    )

        # --- attention + output projection ---
        with (
            tc.tile_pool(name="psST", bufs=2, space="PSUM") as psST,
            tc.tile_pool(name="psOT", bufs=3, space="PSUM") as psOT,
            tc.tile_pool(name="psY", bufs=1, space="PSUM") as psY,
            tc.tile_pool(name="estA", bufs=3) as estA,
            tc.tile_pool(name="estD", bufs=3) as estD,
            tc.tile_pool(name="smallp", bufs=3) as smallp,
            tc.tile_pool(name="yp", bufs=2) as yp,
            tc.tile_pool(name="dramp", bufs=3, space="DRAM") as dramp,
        ):

            def expst(st, idx, dve_set):
                """exp(st*0.125): exact-ACT or Schraudolph-DVE."""
                if idx not in dve_set:
                    est = estA.tile([128, 2, GQ], F16, tag="estA")
                    nc.scalar.activation(
                        est[:], st[:], mybir.ActivationFunctionType.Exp, scale=0.125
                    )
                    return est
                esti = estD.tile([128, 2, GQ], I16, tag="estD")
                nc.vector.tensor_scalar(
                    esti[:],
                    st[:],
                    K_SCH,
                    B_SCH,
                    mybir.AluOpType.mult,
                    mybir.AluOpType.add,
                )
                return esti.bitcast(F16)

            def normalize(ps_ot, ot_dst):
                # recip_approx_fast requires SBUF input (PSUM reads misbehave)
                sd = smallp.tile([1, GQ], F32, tag="sd")
                nc.vector.tensor_copy(sd[:], ps_ot[64:65, :])
                rc = smallp.tile([1, GQ], F32, tag="rc")
                nc.vector.reciprocal_approx_fast(rc[:], sd[:])
                dn = dramp.tile([GQ], F32, tag="dn")
                nc.gpsimd.dma_start(dn[:], rc[:])
                rb = smallp.tile([64, GQ], F32, tag="rb")
                nc.gpsimd.dma_start(rb[:], dn[None, :].to_broadcast((64, GQ)))
                nc.vector.tensor_tensor(
                    ot_dst, ps_ot[0:64, :], rb[:], mybir.AluOpType.mult
                )

            def out_proj(tq, ot_tile, lo=0, hi=GQ // 128):
                for tsub in range(lo, hi):
                    y_sb = yp.tile([128, C], F32, tag="y_sb")
                    for nh in range(2):
                        py = psY.tile([128, 384], F32, tag="y")
                        for h in range(HPC):
                            nc.tensor.matmul(
                                py[:],
                                (ot_tile[:, h, ts(tsub, 128)]),
                                (wout_sb[:, h, ts(nh, 384)]),
                                start=(h == 0),
                                stop=(h == HPC - 1),
                            )
                        nc.vector.tensor_tensor(
                            y_sb[:, ts(nh, 384)],
                            py[:],
                            bout_bc[:, ts(nh, 384)],
                            mybir.AluOpType.add,
                        )
                    nc.sync.dma_start(y3[:, tq * (GQ // 128) + tsub, :], y_sb[:])

            prev = None  # (tq, ot_tile) pending out-projection
            for tq in range(NTQ):
                ot_tile = ot_bufs[tq % 2]

                # -- phase A: heads 0,1 (paired on PE rows lo/hi) --
                # software-pipelined: ST/exp issued 2 iterations ahead of AV
                ps_ot0 = psOT.tile([65, GQ], F32, tag="ot")
                ps_ot1 = psOT.tile([65, GQ], F32, tag="ot")
                ests = {}
                for i in range(NTK + 2):
                    if i < NTK:
                        st = psST.tile([128, 2, GQ], F32, tag="st")
                        nc.tensor.matmul(
                            st[:, 0, :],
                            (qkT[0:64, 1, ts(i, 128)]),
                            (qkT[0:64, 0, ts(tq, GQ)]),
                            start=True,
                            stop=True,
                        )
                        nc.tensor.matmul(
                            st[:, 1, :],
                            (qkT[64:128, 1, ts(i, 128)]),
                            (qkT[64:128, 0, ts(tq, GQ)]),
                            start=True,
                            stop=True,
                        )
                        ests[i] = expst(st, i, DVE_A)
                    if i >= 2:
                        g = i - 2
                        est = ests.pop(g)
                        nc.tensor.matmul(
                            ps_ot0[:],
                            (vaug[:, g, 0, :]),
                            (est[:, 0, :]),
                            start=(g == 0),
                            stop=(g == NTK - 1),
                        )
                        nc.tensor.matmul(
                            ps_ot1[:],
                            (vaug[:, g, 1, :]),
                            (est[:, 1, :]),
                            start=(g == 0),
                            stop=(g == NTK - 1),
                        )
                normalize(ps_ot0, ot_tile[0:64, 0, :])
                normalize(ps_ot1, ot_tile[0:64, 1, :])

                # -- phase B: head 2 (paired across even/odd tk-blocks) --
                ps_ot2 = psOT.tile([65, GQ], F32, tag="ot")
                for i in range(NTK // 2 + 2):
                    # spread the pending out-projection through phase B so
                    # its DVE y-copies don't bunch up at the tq boundary
                    if prev is not None and i in (4, 8, 12, 16):
                        out_proj(prev[0], prev[1], (i - 4) // 4, i // 4)
                    if i < NTK // 2:
                        ge, go = 2 * i, 2 * i + 1
                        st = psST.tile([128, 2, GQ], F32, tag="st")
                        nc.tensor.matmul(
                            st[:, 0, :],
                            (qkT[0:64, 3, ts(ge, 128)]),
                            (qkT[0:64, 2, ts(tq, GQ)]),
                            start=True,
                            stop=True,
                        )
                        nc.tensor.matmul(
                            st[:, 1, :],
                            (qkT[64:128, 2, ts(go, 128)]),
                            (qkT[64:128, 3, ts(tq, GQ)]),
                            start=True,
                            stop=True,
                        )
                        ests[i] = expst(st, i, DVE_B)
                    if i >= 2:
                        g2 = i - 2
                        ge, go = 2 * g2, 2 * g2 + 1
                        est = ests.pop(g2)
                        nc.tensor.matmul(
                            ps_ot2[:],
                            (vaug[:, ge, 2, :]),
                            (est[:, 0, :]),
                            start=(g2 == 0),
                            stop=False,
                        )
                        nc.tensor.matmul(
                            ps_ot2[:],
                            (vaug[:, go, 2, :]),
                            (est[:, 1, :]),
                            start=False,
                            stop=(g2 == NTK // 2 - 1),
                        )
                normalize(ps_ot2, ot_tile[0:64, 2, :])
                prev = (tq, ot_tile)
            out_proj(*prev)

    nc.compile()
    return nc


_PROGRAM = None


def _get_program():
    global _PROGRAM
    if _PROGRAM is None:
        _PROGRAM = _build_program()
    return _PROGRAM


def _make_in_maps(x, W_qkv, b_qkv, W_out, b_out):
    x = np.asarray(x, dtype=np.float32)
    W_qkv = np.asarray(W_qkv, dtype=np.float32)
    b_qkv = np.asarray(b_qkv, dtype=np.float32)
    W_out = np.asarray(W_out, dtype=np.float32)
    b_out = np.asarray(b_out, dtype=np.float32)

    def pmaj(a, nchunk):
        # [nchunk*128, F...] -> partition-major [128, nchunk, F...]
        return np.ascontiguousarray(
            a.reshape(nchunk, 128, *a.shape[1:]).swapaxes(0, 1)
        )

    xT_b = [
        pmaj(np.ascontiguousarray(x[b].T).astype(np.float16), 6) for b in range(B)
    ]
    in_maps = []
    for c in range(N_CORES):
        b, g = divmod(c, 4)
        h0 = HPC * g

        def qcol(h):
            return slice(h * DK, (h + 1) * DK)

        def kcol(h):
            return slice(C + h * DK, C + (h + 1) * DK)

        wqk_c = np.concatenate(
            [
                W_qkv[:, qcol(h0)],
                W_qkv[:, qcol(h0 + 1)],
                W_qkv[:, kcol(h0)],
                W_qkv[:, kcol(h0 + 1)],
                W_qkv[:, qcol(h0 + 2)],
                W_qkv[:, kcol(h0 + 2)],
            ],
            axis=1,
        )
        bqk_c = np.concatenate(
            [
                b_qkv[qcol(h0)],
                b_qkv[qcol(h0 + 1)],
                b_qkv[kcol(h0)],
                b_qkv[kcol(h0 + 1)],
                b_qkv[qcol(h0 + 2)],
                b_qkv[kcol(h0 + 2)],
            ]
        )
        vs = slice(2 * C + h0 * DK, 2 * C + (h0 + HPC) * DK)
        in_maps.append(
            {
                "xT": xT_b[b],
                "wqk": pmaj(wqk_c.astype(np.float16), 6),
                "bqk": pmaj(bqk_c, 3).copy(),
                "wv": pmaj(W_qkv[:, vs].astype(np.float16), 6),
                "bv": np.ascontiguousarray(b_qkv[vs]),
                "wout": np.ascontiguousarray(
                    W_out[h0 * DK : (h0 + HPC) * DK, :]
                ).astype(np.float16),
                "bout": (b_out if g == 0 else np.zeros_like(b_out)).copy(),
            }
        )
    return in_maps


def _assemble(results):
    out = np.zeros((B, T, C), dtype=np.float32)
    for c in range(N_CORES):
        out[c // 4] += results[c]["y"]
    return out


def kernel_run(inputs, trace=False):
    """Returns (full_output [B,T,C] fp32, exec_time_ns or None)."""
    nc = _get_program()
    in_maps = _make_in_maps(**inputs)
    res = run_bass_kernel_spmd(
        nc, in_maps, core_ids=list(range(N_CORES)), trace=trace
    )
    return _assemble(res.results), res.exec_time_ns


def kernel(**inputs):
    out, _ = kernel_run(inputs)
    return out


# revision 26
# speedup vs baseline: 1.7568x; 1.0085x over previous
"""Multi-head attention (B=2, T=4096, C=768, H=12, Dk=64) on 8 trn2 NeuronCores.

Sharding: core c -> batch b = c//4, head-group g = c%4 (3 heads each).
Megatron-style: each core computes qkv projection for its 3 heads, full
attention for those heads, and a row-parallel partial of the output
projection. Host sums the 4 partials per batch (+ bias, folded into the
g==0 core's partial on device).

v4:
  - exp split ~0.35 to DVE (Schraudolph f16-bit, mean-zero offset), with
    DVE's share scheduled mid-phase so it never collides with the
    normalize/y-copy DVE bursts at tq boundaries.
  - out-projection packed into two [128,128]-stationary matmuls (h0|h1
    stacked via an SBUF partition-move DMA; h2 zero-padded) - the 64-row
    stationary form can't use the fast weight-load path.
  - softmax denominator row copied out of PSUM on ACT; reciprocal via
    reciprocal_approx_fast (SBUF input only - PSUM reads misbehave).
  - x & W_qk f16; x loaded once in 4 chunks on 2 DMA queues.
  - software-pipelined attention (ST 2 ahead of AV) + out-projection one
    tq behind; normalize roundtrips and o3-swaps on the gpsimd queue.
"""

import os
import sys
from contextlib import ExitStack

import numpy as np

for _p in ("/opt/trn_rl_repo", "/root/.axon_site/_ro/trn_rl_repo"):
    if os.path.isdir(_p) and _p not in sys.path:
        sys.path.append(_p)

import concourse.bass as bass
import concourse.mybir as mybir
import concourse.tile as tile
from concourse import bacc
from concourse.bass import ts
from concourse.bass_utils import run_bass_kernel_spmd

F32 = mybir.dt.float32
F16 = mybir.dt.float16
I16 = mybir.dt.int16

B, T, C = 2, 4096, 768
H, DK = 12, 64
N_CORES = 8
HPC = 3  # heads per core
GQ = 512  # q-block (matmul free dim)
NTQ = T // GQ  # 8 q-blocks
NTK = T // 128  # 32 tk-blocks

# Schraudolph f16-bit exp: exp(s/8) ~ bits_f16(round(s*K + B)), mean-zero
# offset so the approx blocks carry no systematic bias vs the exact blocks
# sharing the same softmax denominator.
K_SCH = 1024.0 * float(np.log2(np.e)) / 8.0
B_SCH = 15308.5
# which ST blocks take the DVE-approx exp (mid-phase only: the DVE is busy
# with normalize + y copies around tq boundaries)
DVE_A = {8, 10, 12, 14, 16, 18, 20, 22, 24, 26, 28}
DVE_B = {4, 6, 8, 10, 12, 14}


def _build_program():
    nc = bacc.Bacc("TRN2", target_bir_lowering=False, debug=False)

    # xT/wqk/wv/bqk come host-pre-arranged partition-major: loads are one
    # contiguous run per partition (128 DMA descriptors, not 768)
    xT = nc.dram_tensor("xT", [128, 6, T], F16, kind="ExternalInput").ap()
    wqk = nc.dram_tensor("wqk", [128, 6, 384], F16, kind="ExternalInput").ap()
    bqk = nc.dram_tensor("bqk", [128, 3], F32, kind="ExternalInput").ap()
    wv = nc.dram_tensor("wv", [128, 6, 192], F16, kind="ExternalInput").ap()
    bv = nc.dram_tensor("bv", [192], F32, kind="ExternalInput").ap()
    wout = nc.dram_tensor("wout", [192, C], F16, kind="ExternalInput").ap()
    bout = nc.dram_tensor("bout", [C], F32, kind="ExternalInput").ap()
    y = nc.dram_tensor("y", [T, C], F32, kind="ExternalOutput").ap()

    y3 = y.rearrange("(n p) e -> p n e", p=128)  # [128, 32, 768]

    with tile.TileContext(nc) as tc, ExitStack() as ctx:
        sb = ctx.enter_context(tc.tile_pool(name="persist", bufs=1))

        # --- weights / biases first (small), then x in 4 chunks / 2 queues ---
        wqk_sb = sb.tile([128, 6, 384], F16)
        nc.sync.dma_start(wqk_sb[:], wqk)
        wv_sb = sb.tile([128, 6, 192], F16)
        nc.sync.dma_start(wv_sb[:], wv)
        # out-proj weights: one [128,128]-stationary slot per head, rows
        # 64:128 zero-padded (written once; [64,x] stationaries can't use
        # the fast weight-load path)
        wout_sb = sb.tile([128, 3, C], F16)
        for h in range(HPC):
            nc.gpsimd.dma_start(wout_sb[0:64, h, :], wout[ts(h, 64), :])
        nc.vector.memset(wout_sb[64:128, :, :], 0.0)
        bqk_sb = sb.tile([128, 3], F32)
        nc.sync.dma_start(bqk_sb[:], bqk)
        bv_bc = sb.tile([128, 192], F32)
        nc.sync.dma_start(bv_bc[:], bv[None, :].to_broadcast((128, 192)))
        bout_bc = sb.tile([128, C], F32)
        nc.sync.dma_start(bout_bc[:], bout[None, :].to_broadcast((128, C)))

        # x split o-wise over 3 DMA queues (sync/gpsimd/scalar), first-needed
        # chunks first, so the opening qk matmuls aren't transfer-bound
        x16 = sb.tile([128, 6, T], F16)
        x_engs = (nc.sync, nc.gpsimd, nc.scalar)
        for o in range(6):
            x_engs[o % 3].dma_start(x16[:, o : o + 1, :], xT[:, o : o + 1, :])

        # --- persistent activations ---
        # qkT slots: o0=[q0|q1] o1=[k0|k1] o2=[q2|k2] o3=[k2|q2]
        qkT = sb.tile([128, 4, T], F16)
        # V per tk-block per head, token-major, with ones col at [.., 64]
        vaug = sb.tile([128, NTK, HPC, 65], F16)
        nc.vector.memset(vaug[:, :, :, 64:65], 1.0)
        # normalized attention outputs, double-buffered across tq; rows
        # 64:128 zero-padded once to match the padded wout stationaries
        ot_bufs = []
        for bi in range(2):
            otb = sb.tile([128, HPC, GQ], F16, name=f"otb{bi}")
            nc.vector.memset(otb[64:128, :, :], 0.0)
            ot_bufs.append(otb)

        # --- prologue: qkv projections ---
        with (
            tc.tile_pool(name="psQK", bufs=2, space="PSUM") as psQK,
            tc.tile_pool(name="psV", bufs=2, space="PSUM") as psV,
        ):
            for tb in range(NTQ):
                for cb in range(3):
                    ps = psQK.tile([128, GQ], F32, tag="qk")
                    for d in range(6):
                        nc.tensor.matmul(
                            ps[:],
                            (wqk_sb[:, d, ts(cb, 128)]),
                            (x16[:, d, ts(tb, GQ)]),
                            start=(d == 0),
                            stop=(d == 5),
                        )
                    # bias-add + f16 copy on ACT (idle in prologue)
                    nc.scalar.activation(
                        qkT[:, cb, ts(tb, GQ)],
                        ps[:],
                        mybir.ActivationFunctionType.Identity,
                        bias=bqk_sb[:, cb : cb + 1],
                    )
                # o3 = swap halves of o2 (k2|q2)
                nc.gpsimd.dma_start(
                    qkT[0:64, 3, ts(tb, GQ)], qkT[64:128, 2, ts(tb, GQ)]
                )
                nc.gpsimd.dma_start(
                    qkT[64:128, 3, ts(tb, GQ)], qkT[0:64, 2, ts(tb, GQ)]
                )

                # v-projection (f16), 128-token sub-blocks
                for sub in range(GQ // 128):
                    g = tb * (GQ // 128) + sub
                    psv = psV.tile([128, 192], F32, tag="v")
                    for d in range(6):
                        nc.tensor.matmul(
                            psv[:],
                            (x16[:, d, ts(g, 128)]),
                            (wv_sb[:, d, :]),
                            start=(d == 0),
                            stop=(d == 5),
                        )
                    nc.vector.tensor_tensor(
                        vaug[:, g, :, 0:64],
                        psv[:].rearrange("p (h d) -> p h d", d=64),
                        bv_bc[:].rearrange("p (h d) -> p h d", d=64),
                        mybir.AluOpType.add,
                    )

        # --- attention + output projection ---
        with (
            tc.tile_pool(name="psST", bufs=2, space="PSUM") as psST,
            tc.tile_pool(name="psOT", bufs=3, space="PSUM") as psOT,
            tc.tile_pool(name="psY", bufs=1, space="PSUM") as psY,
            tc.tile_pool(name="estA", bufs=4) as estA,
            tc.tile_pool(name="estD", bufs=4) as estD,
            tc.tile_pool(name="smallp", bufs=3) as smallp,
            tc.tile_pool(name="yp", bufs=2) as yp,
            tc.tile_pool(name="dramp", bufs=3, space="DRAM") as dramp,
        ):

            def expst(st, idx, dve_set):
                """exp(st*0.125): exact-ACT or Schraudolph-DVE."""
                if idx not in dve_set:
                    est = estA.tile([128, 2, GQ], F16, tag="estA")
                    nc.scalar.activation(
                        est[:], st[:], mybir.ActivationFunctionType.Exp, scale=0.125
                    )
                    return est
                esti = estD.tile([128, 2, GQ], I16, tag="estD")
                nc.vector.tensor_scalar(
                    esti[:],
                    st[:],
                    K_SCH,
                    B_SCH,
                    mybir.AluOpType.mult,
                    mybir.AluOpType.add,
                )
                return esti.bitcast(F16)

            def normalize(ps_ot, ot_dst):
                # recip_approx_fast requires SBUF input (PSUM reads misbehave)
                sd = smallp.tile([1, GQ], F32, tag="sd")
                nc.vector.tensor_copy(sd[:], ps_ot[64:65, :])
                rc = smallp.tile([1, GQ], F32, tag="rc")
                nc.vector.reciprocal_approx_fast(rc[:], sd[:])
                dn = dramp.tile([GQ], F32, tag="dn")
                nc.gpsimd.dma_start(dn[:], rc[:])
                rb = smallp.tile([64, GQ], F32, tag="rb")
                nc.gpsimd.dma_start(rb[:], dn[None, :].to_broadcast((64, GQ)))
                nc.vector.tensor_tensor(
                    ot_dst, ps_ot[0:64, :], rb[:], mybir.AluOpType.mult
                )

            def out_proj(tq, ot_tile, lo=0, hi=GQ // 128):
                for tsub in range(lo, hi):
                    y_sb = yp.tile([128, C], F32, tag="y_sb")
                    for nh in range(2):
                        py = psY.tile([128, 384], F32, tag="y")
                        for h in range(HPC):
                            nc.tensor.matmul(
                                py[:],
                                (ot_tile[:, h, ts(tsub, 128)]),
                                (wout_sb[:, h, ts(nh, 384)]),
                                start=(h == 0),
                                stop=(h == HPC - 1),
                            )
                        nc.vector.tensor_tensor(
                            y_sb[:, ts(nh, 384)],
                            py[:],
                            bout_bc[:, ts(nh, 384)],
                            mybir.AluOpType.add,
                        )
                    nc.sync.dma_start(y3[:, tq * (GQ // 128) + tsub, :], y_sb[:])

            prev = None  # (tq, ot_tile) pending out-projection
            for tq in range(NTQ):
                ot_tile = ot_bufs[tq % 2]

                # -- phase A: heads 0,1 (paired on PE rows lo/hi) --
                # software-pipelined: ST/exp issued 2 iterations ahead of AV
                ps_ot0 = psOT.tile([65, GQ], F32, tag="ot")
                ps_ot1 = psOT.tile([65, GQ], F32, tag="ot")
                def st_a(i):
                    st = psST.tile([128, 2, GQ], F32, tag="st", name="st")
                    nc.tensor.matmul(
                        st[:, 0, :],
                        (qkT[0:64, 1, ts(i, 128)]),
                        (qkT[0:64, 0, ts(tq, GQ)]),
                        start=True,
                        stop=True,
                    )
                    nc.tensor.matmul(
                        st[:, 1, :],
                        (qkT[64:128, 1, ts(i, 128)]),
                        (qkT[64:128, 0, ts(tq, GQ)]),
                        start=True,
                        stop=True,
                    )
                    return st

                def av_a(g, est):
                    nc.tensor.matmul(
                        ps_ot0[:],
                        (vaug[:, g, 0, :]),
                        (est[:, 0, :]),
                        start=(g == 0),
                        stop=(g == NTK - 1),
                    )
                    nc.tensor.matmul(
                        ps_ot1[:],
                        (vaug[:, g, 1, :]),
                        (est[:, 1, :]),
                        start=(g == 0),
                        stop=(g == NTK - 1),
                    )

                # g processed in pairs: the two ST row-pair groups issue
                # back-to-back so each pair's weight loads hide under the
                # other half's matmul
                ests = {}
                for i in range(0, NTK + 4, 2):
                    if i < NTK:
                        ests[i] = expst(st_a(i), i, DVE_A)
                        ests[i + 1] = expst(st_a(i + 1), i + 1, DVE_A)
                    if i >= 4:
                        av_a(i - 4, ests.pop(i - 4))
                        av_a(i - 3, ests.pop(i - 3))
                normalize(ps_ot0, ot_tile[0:64, 0, :])
                normalize(ps_ot1, ot_tile[0:64, 1, :])

                # -- phase B: head 2 (paired across even/odd tk-blocks) --
                ps_ot2 = psOT.tile([65, GQ], F32, tag="ot")

                def st_b(i):
                    st = psST.tile([128, 2, GQ], F32, tag="st", name="st")
                    nc.tensor.matmul(
                        st[:, 0, :],
                        (qkT[0:64, 3, ts(2 * i, 128)]),
                        (qkT[0:64, 2, ts(tq, GQ)]),
                        start=True,
                        stop=True,
                    )
                    nc.tensor.matmul(
                        st[:, 1, :],
                        (qkT[64:128, 2, ts(2 * i + 1, 128)]),
                        (qkT[64:128, 3, ts(tq, GQ)]),
                        start=True,
                        stop=True,
                    )
                    return st

                def av_b(g2, est):
                    nc.tensor.matmul(
                        ps_ot2[:],
                        (vaug[:, 2 * g2, 2, :]),
                        (est[:, 0, :]),
                        start=(g2 == 0),
                        stop=False,
                    )
                    nc.tensor.matmul(
                        ps_ot2[:],
                        (vaug[:, 2 * g2 + 1, 2, :]),
                        (est[:, 1, :]),
                        start=False,
                        stop=(g2 == NTK // 2 - 1),
                    )

                for i in range(0, NTK // 2 + 4, 2):
                    # spread the pending out-projection through phase B so
                    # its DVE y-copies don't bunch up at the tq boundary
                    if prev is not None and i in (4, 8, 12, 16):
                        out_proj(prev[0], prev[1], (i - 4) // 4, i // 4)
                    if i < NTK // 2:
                        ests[i] = expst(st_b(i), i, DVE_B)
                        ests[i + 1] = expst(st_b(i + 1), i + 1, DVE_B)
                    if i >= 4:
                        av_b(i - 4, ests.pop(i - 4))
                        av_b(i - 3, ests.pop(i - 3))
                normalize(ps_ot2, ot_tile[0:64, 2, :])
                prev = (tq, ot_tile)
            out_proj(*prev)

    nc.compile()
    return nc


_PROGRAM = None


def _get_program():
    global _PROGRAM
    if _PROGRAM is None:
        _PROGRAM = _build_program()
    return _PROGRAM


def _make_in_maps(x, W_qkv, b_qkv, W_out, b_out):
    x = np.asarray(x, dtype=np.float32)
    W_qkv = np.asarray(W_qkv, dtype=np.float32)
    b_qkv = np.asarray(b_qkv, dtype=np.float32)
    W_out = np.asarray(W_out, dtype=np.float32)
    b_out = np.asarray(b_out, dtype=np.float32)

    def pmaj(a, nchunk):
        # [nchunk*128, F...] -> partition-major [128, nchunk, F...]
        return np.ascontiguousarray(
            a.reshape(nchunk, 128, *a.shape[1:]).swapaxes(0, 1)
        )

    xT_b = [
        pmaj(np.ascontiguousarray(x[b].T).astype(np.float16), 6) for b in range(B)
    ]
    in_maps = []
    for c in range(N_CORES):
        b, g = divmod(c, 4)
        h0 = HPC * g

        def qcol(h):
            return slice(h * DK, (h + 1) * DK)

        def kcol(h):
            return slice(C + h * DK, C + (h + 1) * DK)

        wqk_c = np.concatenate(
            [
                W_qkv[:, qcol(h0)],
                W_qkv[:, qcol(h0 + 1)],
                W_qkv[:, kcol(h0)],
                W_qkv[:, kcol(h0 + 1)],
                W_qkv[:, qcol(h0 + 2)],
                W_qkv[:, kcol(h0 + 2)],
            ],
            axis=1,
        )
        bqk_c = np.concatenate(
            [
                b_qkv[qcol(h0)],
                b_qkv[qcol(h0 + 1)],
                b_qkv[kcol(h0)],
                b_qkv[kcol(h0 + 1)],
                b_qkv[qcol(h0 + 2)],
                b_qkv[kcol(h0 + 2)],
            ]
        )
        vs = slice(2 * C + h0 * DK, 2 * C + (h0 + HPC) * DK)
        in_maps.append(
            {
                "xT": xT_b[b],
                "wqk": pmaj(wqk_c.astype(np.float16), 6),
                "bqk": pmaj(bqk_c, 3).copy(),
                "wv": pmaj(W_qkv[:, vs].astype(np.float16), 6),
                "bv": np.ascontiguousarray(b_qkv[vs]),
                "wout": np.ascontiguousarray(
                    W_out[h0 * DK : (h0 + HPC) * DK, :]
                ).astype(np.float16),
                "bout": (b_out if g == 0 else np.zeros_like(b_out)).copy(),
            }
        )
    return in_maps


def _assemble(results):
    out = np.zeros((B, T, C), dtype=np.float32)
    for c in range(N_CORES):
        out[c // 4] += results[c]["y"]
    return out


def kernel_run(inputs, trace=False):
    """Returns (full_output [B,T,C] fp32, exec_time_ns or None)."""
    nc = _get_program()
    in_maps = _make_in_maps(**inputs)
    res = run_bass_kernel_spmd(
        nc, in_maps, core_ids=list(range(N_CORES)), trace=trace
    )
    return _assemble(res.results), res.exec_time_ns


def kernel(**inputs):
    out, _ = kernel_run(inputs)
    return out
